# revision 4
# baseline (speedup 1.0000x reference)
"""Grouped-query attention (B=1, S=2048, HID=4096, 32 q-heads / 8 kv-heads,
D=128, RoPE, additive causal mask) on 8 Trainium2 NeuronCores.

Sharding (tensor-parallel over heads, per the sharding hint): core c owns 4
q-heads (columns 512c:512c+512 of Wq), kv-head c (columns 128c:128c+128 of
Wk/Wv), and rows 512c:512c+512 of Wo. Each core emits a full-shape partial
of the output projection; the host sums the 8 partials (the "all-reduce" of
the row-sharded Wo matmul).

Design notes (TimelineSim ~388us/core vs 606us baseline):
  - All matmuls bf16 (1 PE cycle/row like f32r, half the DMA/SBUF; fp8 was
    measured numerically unacceptable for the 2e-2 gate). Weights are
    host-prepacked into their exact SBUF layouts so every DMA moves large
    contiguous descriptors.
  - One unified PSUM scheme: four [128,1024] double-bank tiles (P0..P3)
    tag-rotated through warmup / QKV+RoPE / attention / output projection.
    No psum pool open/close between phases, so there are no cross-phase
    allocation barriers - a new use of a tag only waits for that tag's
    previous consumers.
  - PE warmup chain at t~0 (tiny matmuls on a zeroed tile) so the tensor
    engine p-state ramp (0.65/1.2/2.4 GHz) is complete and the PE never
    idles while the first weight/x DMAs land.
  - Causal masking is free-form: score tiles are computed at variable width
    (only columns right of the diagonal), exp'd unmasked, and the single
    boundary 128-block is multiplied by a constant lower-triangular 0/1
    bf16 mask on the DVE (2x mode). No mask DMA, no PSUM mask adds.
    A generic additive-mask fallback is built if the host detects a
    non-causal attention_mask input.
  - Attention is one software-pipelined stream of k-tile PAIRS: two k-tiles
    share a [128,1024] psum score tile and (off-diagonal) a single exp, so
    the Act engine's fixed per-op overhead is halved; lsum(ones matmul) and
    PV accumulate per tile with interleaved accumulation groups.
  - V is produced directly in [s, d] layout (x chunk as the stationary
    operand) - no PE transposes. Q/K run feature-major with RoPE applied by
    the DVE in 2x bf16 mode from an Act-drained copy of the psum (the
    sign-folded sin table is pre-rolled 64 partitions so both SBUF operands
    of each half-multiply share a base partition, a BIR requirement).
  - Softmax normalization: DVE reciprocal of the lsum row, Pool
    partition_broadcast, one DVE multiply into the bf16 outT tile.
  - Output projection drains psum via two independent [128,512] copies
    (alternating Act/DVE) so each half's y DMA fires as soon as its copy
    lands; y is written bf16 and the partials are summed f32 on the host.
"""
import os

import numpy as np
import ml_dtypes
from contextlib import ExitStack

import concourse.bass as bass
import concourse.tile as tile
from concourse import bacc, mybir
from concourse.bass_utils import run_bass_kernel_spmd

F32 = mybir.dt.float32
BF16 = mybir.dt.bfloat16
EXP = mybir.ActivationFunctionType.Exp
NPBF16 = ml_dtypes.bfloat16

S = 2048
HID = 4096
D = 128
NCORES = 8
NHQ = 4                      # q heads per core
SCALE = float(D) ** -0.5
ST = S // 128                # 16 s-tiles
SL = S // 512                # 4 s-slices
KT = HID // 128              # 32 hidden k-tiles
NO = HID // 512              # 8 output column slices
NWARM = 90                   # PE warmup matmuls (bridges first DMAs)

_NC_CACHE = {}


def build_nc(generic_mask=False, nwarm=None, debug_taps=False):
    nc = bacc.Bacc("TRN2", target_bir_lowering=False, debug=False,
                   num_devices=NCORES)
    xT = nc.dram_tensor("xT", [HID, S], BF16, kind="ExternalInput").ap()
    wq_pre = nc.dram_tensor("wq_pre", [8, 128, 4, 512], BF16,
                            kind="ExternalInput").ap()
    wkv_pre = nc.dram_tensor("wkv_pre", [8, 128, 4, 256], BF16,
                             kind="ExternalInput").ap()
    wo_pre = nc.dram_tensor("wo_pre", [128, NHQ, HID], BF16,
                            kind="ExternalInput").ap()
    cosT = nc.dram_tensor("cosT", [128, S], BF16, kind="ExternalInput").ap()
    sinTf = nc.dram_tensor("sinTf", [128, S], BF16, kind="ExternalInput").ap()
    ones = nc.dram_tensor("ones", [128, 1], BF16, kind="ExternalInput").ap()
    if generic_mask:
        maskT = nc.dram_tensor("maskT", [ST, 128, 512], BF16,
                               kind="ExternalInput").ap()
    y = nc.dram_tensor("y", [S, HID], BF16, kind="ExternalOutput").ap()
    if debug_taps:
        qt_d = nc.dram_tensor("qt_d", [NHQ, 128, S], BF16,
                              kind="ExternalOutput").ap()
        kt_d = nc.dram_tensor("kt_d", [128, S], BF16,
                              kind="ExternalOutput").ap()
        v_d = nc.dram_tensor("v_d", [128, ST, 128], BF16,
                             kind="ExternalOutput").ap()
        o_d = nc.dram_tensor("o_d", [NHQ, 128, S], BF16,
                             kind="ExternalOutput").ap()

    with tile.TileContext(nc) as tc, ExitStack() as ctx:
        const = ctx.enter_context(tc.tile_pool(name="const", bufs=1))
        cos_sb = const.tile([128, S], BF16)
        sin_sb = const.tile([128, S], BF16)
        ones_sb = const.tile([128, 1], BF16)
        qt = [const.tile([128, S], BF16, tag=f"qt{h}", name=f"qt{h}")
              for h in range(NHQ)]
        kt = const.tile([128, S], BF16)
        v_sb = const.tile([128, ST, 128], BF16)
        outT = [const.tile([128, S], BF16, tag=f"outT{h}", name=f"outT{h}")
                for h in range(NHQ)]
        tri = const.tile([128, 128], BF16)
        if generic_mask:
            mask_sb = const.tile([128, ST, 512], BF16)

        # Unified PSUM: four 2-bank [128,1024] tiles, tag-rotated through all
        # phases. No pool open/close -> no cross-phase handover barriers; a
        # new instance of tag Pn only waits for the previous Pn consumers.
        psum = ctx.enter_context(tc.tile_pool(name="psum", bufs=1,
                                              space="PSUM"))

        def ptile(tag):
            return psum.tile([128, 1024], F32, tag=tag, name=tag)

        # ---------------- PE warmup: own the clock ramp from t~0 ------------
        warm = ctx.enter_context(tc.tile_pool(name="warm", bufs=1))
        wsb = warm.tile([128, 64], BF16)
        nc.gpsimd.memset(wsb[:], 0.0)
        wps = ptile("P3")
        for _ in range(nwarm if nwarm is not None else NWARM):
            nc.tensor.matmul(wps[0:64, 0:64], wsb[:, 0:64], wsb[:, 0:64],
                             start=True, stop=True, skip_group_check=True)
        nc.gpsimd.memset(tri[:], 1.0)
        nc.gpsimd.affine_select(out=tri[:], in_=tri[:],
                                compare_op=mybir.AluOpType.is_ge, fill=0.0,
                                base=0, pattern=[[1, 128]],
                                channel_multiplier=-1)

        # ---------------- Phase A: projections + RoPE -----------------------
        wA = ctx.enter_context(tc.tile_pool(name="wA", bufs=1))
        wq_sb = wA.tile([128, KT, 512], BF16)
        wkv_sb = wA.tile([128, KT, 256], BF16)
        wo_sb = wA.tile([128, NHQ, HID], BF16)

        with tc.tile_pool(name="xtp", bufs=4) as xtp, \
             tc.tile_pool(name="xtp2", bufs=4) as xtp2, \
             tc.tile_pool(name="drains", bufs=2) as drains, \
             tc.tile_pool(name="ropes", bufs=2) as ropes:

            def rope_from(dst_slice, sb, cs, sn):
                """dst = sb*cos + rotate_half(sb)*sin; all-bf16 -> DVE 2x.

                sn is the sign-folded sin table rolled by 64 partitions, so
                each half-multiply reads both SBUF operands at the SAME base
                partition (BIR requires equal input bases for SB+SB ops).
                """
                rot = ropes.tile([128, 512], BF16, tag="rot")
                nc.vector.tensor_mul(rot[0:64, :], sb[64:128, :],
                                     sn[64:128, :])
                nc.vector.tensor_mul(rot[64:128, :], sb[0:64, :], sn[0:64, :])
                nc.vector.tensor_mul(dst_slice, sb[:, :], cs)
                nc.vector.tensor_add(dst_slice, dst_slice, rot[:])

            for j in range(SL):
                # P0 = [q0|q1], P1 = [q2|q3], P2 = [k|v]
                acc = [ptile("P0"), ptile("P1"), ptile("P2")]
                qps = [acc[0][:, 0:512], acc[0][:, 512:1024],
                       acc[1][:, 0:512], acc[1][:, 512:1024]]
                kps = acc[2][:, 0:512]
                vtile = acc[2][:, 512:1024]
                for g in range(8):
                    if j == 0:
                        # weights ahead of x in queue order; g=0 in halves so
                        # the first matmul group can fire earlier
                        xt = xtp.tile([128, 4, 512], BF16, tag="xt")
                        xts = xt
                        for hh in ([0, 1] if g == 0 else [None]):
                            if hh is None:
                                ksl = slice(4 * g, 4 * g + 4)
                                kk2 = slice(0, 4)
                            else:
                                ksl = slice(2 * hh, 2 * hh + 2)
                                kk2 = slice(2 * hh, 2 * hh + 2)
                            nc.sync.dma_start(out=wq_sb[:, ksl, :],
                                              in_=wq_pre[g][:, kk2, :])
                            nc.sync.dma_start(out=wkv_sb[:, ksl, :],
                                              in_=wkv_pre[g][:, kk2, :])
                            nc.sync.dma_start(
                                out=xt[:, kk2, :],
                                in_=xT[512 * g + 256 * (hh or 0):
                                       512 * g + 256 * (hh or 0) +
                                       (512 if hh is None else 256),
                                       512 * j:512 * (j + 1)]
                                .rearrange("(t p) m -> p t m", p=128))
                        if g == 7:
                            nc.sync.dma_start(out=cos_sb[:], in_=cosT[:])
                            nc.sync.dma_start(out=sin_sb[:], in_=sinTf[:])
                            nc.sync.dma_start(out=ones_sb[:], in_=ones[:])
                            if generic_mask:
                                nc.sync.dma_start(
                                    out=mask_sb[:],
                                    in_=maskT.rearrange("t p q -> p t q"))
                    else:
                        if g % 2 == 0:
                            xt = xtp2.tile([128, 8, 512], BF16, tag="xt2")
                            nc.sync.dma_start(
                                out=xt[:],
                                in_=xT[1024 * (g // 2):1024 * (g // 2 + 1),
                                       512 * j:512 * (j + 1)]
                                .rearrange("(t p) m -> p t m", p=128))
                        xts = xt[:, 4 * (g % 2):4 * (g % 2 + 1), :]
                    if j == 2 and g < 4:
                        nc.sync.dma_start(out=wo_sb[:, g, :],
                                          in_=wo_pre[:, g, :])
                    for kk in range(4):
                        k = 4 * g + kk
                        st, sp = (k == 0), (k == KT - 1)
                        rhs = xts[:, kk, :]
                        for f in range(NHQ):
                            nc.tensor.matmul(
                                qps[f], wq_sb[:, k, 128 * f:128 * (f + 1)],
                                rhs, start=st, stop=sp,
                                skip_group_check=True)
                        nc.tensor.matmul(kps, wkv_sb[:, k, 0:128], rhs,
                                         start=st, stop=sp,
                                         skip_group_check=True)
                        # V in [s, d] layout: x chunk is the stationary
                        # side. start only on sb_==0: PSUM start zeroing is
                        # 2KB-region wide, so one start covers the whole
                        # vtile bank; per-sb_ starts would wipe each other's
                        # k=0 contribution.
                        for sb_ in range(4):
                            nc.tensor.matmul(
                                vtile[:, 128 * sb_:128 * (sb_ + 1)],
                                xts[:, kk, 128 * sb_:128 * (sb_ + 1)],
                                wkv_sb[:, k, 128:256],
                                start=st and sb_ == 0, stop=sp,
                                skip_group_check=True)
                cs = cos_sb[:, 512 * j:512 * (j + 1)]
                sn = sin_sb[:, 512 * j:512 * (j + 1)]
                # paired Act drains free each 2-bank tile with one copy
                dr0 = drains.tile([128, 1024], BF16, tag="dr0")
                nc.scalar.copy(dr0[:], acc[0][:])
                dr1 = drains.tile([128, 1024], BF16, tag="dr1")
                nc.scalar.copy(dr1[:], acc[1][:])
                drk = drains.tile([128, 512], BF16, tag="drk")
                nc.scalar.copy(drk[:], kps)
                nc.scalar.copy(v_sb[:, 4 * j:4 * (j + 1), :], vtile[:, :])
                rope_from(qt[0][:, 512 * j:512 * (j + 1)], dr0[:, 0:512],
                          cs, sn)
                rope_from(qt[1][:, 512 * j:512 * (j + 1)], dr0[:, 512:1024],
                          cs, sn)
                rope_from(qt[2][:, 512 * j:512 * (j + 1)], dr1[:, 0:512],
                          cs, sn)
                rope_from(qt[3][:, 512 * j:512 * (j + 1)], dr1[:, 512:1024],
                          cs, sn)
                rope_from(kt[:, 512 * j:512 * (j + 1)], drk, cs, sn)

        # ---------------- Phase B: attention --------------------------------
        # One continuous, globally software-pipelined stream of score pairs
        # across all (h, j): sc of pair i+1 overlaps exp/select of pair i,
        # including across (h, j) boundaries, so Act latency never starves PE.
        # scp pairs rotate P3/P0; the [pv | lsum] tile alternates P1/P2.
        with tc.tile_pool(name="ptbp", bufs=4) as ptbp, \
             tc.tile_pool(name="rbcp", bufs=2) as rbcp:
            work = []          # (h, j, ta, tb) in stream order
            for h in range(NHQ):
                for j in range(SL):
                    for p in range((4 * j + 4) // 2):
                        work.append((h, j, 2 * p, 2 * p + 1))

            po = {}
            pts = {}

            def emit_sc(i):
                h, j, ta, tb = work[i]
                if (h, j) not in po:
                    po[(h, j)] = ptile("P1" if (4 * h + j) % 2 == 0 else "P2")

                def off(t):
                    return max(0, 128 * (t - 4 * j))

                oa, ob = off(ta), off(tb)
                scp = ptile("P3" if i % 2 == 0 else "P0")
                nc.tensor.matmul(
                    scp[:, oa:512], kt[:, 128 * ta:128 * (ta + 1)],
                    qt[h][:, 512 * j + oa:512 * (j + 1)],
                    start=True, stop=True, skip_group_check=True)
                nc.tensor.matmul(
                    scp[:, 512 + ob:1024], kt[:, 128 * tb:128 * (tb + 1)],
                    qt[h][:, 512 * j + ob:512 * (j + 1)],
                    start=True, stop=True, skip_group_check=True)
                if generic_mask and tb >= 4 * j:
                    if ta >= 4 * j:
                        nc.vector.tensor_add(scp[:, oa:512], scp[:, oa:512],
                                             mask_sb[:, ta, oa:])
                    nc.vector.tensor_add(scp[:, 512 + ob:1024],
                                         scp[:, 512 + ob:1024],
                                         mask_sb[:, tb, ob:])
                if ta == 0:
                    # first pair of an (h, j): separate tiles so ls/pv of ta
                    # only waits one 512-wide exp, not the whole pair
                    pta = ptbp.tile([128, 512], BF16, tag="ptbf",
                                    name="pta")
                    ptb2 = ptbp.tile([128, 512], BF16, tag="ptbf",
                                     name="ptb2")
                    nc.scalar.activation(pta[:, oa:], scp[:, oa:512],
                                         EXP, bias=0.0, scale=SCALE)
                    nc.scalar.activation(ptb2[:, ob:], scp[:, 512 + ob:1024],
                                         EXP, bias=0.0, scale=SCALE)
                    if not generic_mask:
                        if ta >= 4 * j:
                            nc.vector.tensor_mul(pta[:, oa:oa + 128],
                                                 pta[:, oa:oa + 128], tri[:])
                        if tb >= 4 * j:
                            nc.vector.tensor_mul(ptb2[:, ob:ob + 128],
                                                 ptb2[:, ob:ob + 128],
                                                 tri[:])
                    pts[(h, j, ta)] = pta[:, oa:]
                    pts[(h, j, tb)] = ptb2[:, ob:]
                    return
                ptb = ptbp.tile([128, 1024], BF16, tag="ptb")
                if ta >= 4 * j:          # diagonal pair: two exps
                    nc.scalar.activation(ptb[:, oa:512], scp[:, oa:512],
                                         EXP, bias=0.0, scale=SCALE)
                    nc.scalar.activation(ptb[:, 512 + ob:1024],
                                         scp[:, 512 + ob:1024], EXP,
                                         bias=0.0, scale=SCALE)
                else:                    # one exp across the pair
                    nc.scalar.activation(ptb[:, oa:1024], scp[:, oa:1024],
                                         EXP, bias=0.0, scale=SCALE)
                if not generic_mask:
                    for tx, ox, base in ((ta, oa, 0), (tb, ob, 512)):
                        if tx >= 4 * j:
                            # staircase confined to the first valid block:
                            # multiply by the const lower-tri mask (DVE 2x)
                            nc.vector.tensor_mul(
                                ptb[:, base + ox:base + ox + 128],
                                ptb[:, base + ox:base + ox + 128], tri[:])
                pts[(h, j, ta)] = ptb[:, oa:512]
                pts[(h, j, tb)] = ptb[:, 512 + ob:1024]

            def emit_lspv(i):
                h, j, ta, tb = work[i]
                tmax = 4 * j + 4
                p = po[(h, j)]
                ops, lps = p[:, 0:512], p[0:1, 512:1024]
                for u in (ta, tb):
                    o = max(0, 128 * (u - 4 * j))
                    pu = pts.pop((h, j, u))
                    nc.tensor.matmul(
                        lps[:, o:512], ones_sb[:], pu,
                        start=(u == 0), stop=(u == tmax - 1),
                        skip_group_check=True)
                    nc.tensor.matmul(
                        ops[:, o:512], v_sb[:, u, :], pu,
                        start=(u == 0), stop=(u == tmax - 1),
                        skip_group_check=True)
                if tb == tmax - 1:       # (h, j) complete
                    pending_norm.append((h, j))

            def emit_norm():
                h, j = pending_norm.pop(0)
                p = po[(h, j)]
                ops, lps = p[:, 0:512], p[0:1, 512:1024]
                # stage 1/l into SBUF (DVE reciprocal can read PSUM; GPSIMD
                # cannot), broadcast on Pool, one DVE multiply
                rsb = rbcp.tile([1, 512], F32, tag="rsb")
                nc.vector.reciprocal(rsb[:], lps[:, 0:512])
                rb = rbcp.tile([128, 512], F32, tag="rb")
                nc.gpsimd.partition_broadcast(rb[:], rsb[:])
                nc.vector.tensor_mul(
                    outT[h][:, 512 * j:512 * (j + 1)], ops, rb[:])
                del po[(h, j)]

            pending_norm = []
            for i in range(len(work) + 2):
                if i < len(work):
                    emit_sc(i)
                if 1 <= i <= len(work):
                    emit_lspv(i - 1)
                # normalize deferred one step so its DVE ops rank behind the
                # next iteration's critical tri-mask multiplies
                if pending_norm and (i >= len(work) or
                                     work[i - 1][1] != pending_norm[0][1] or
                                     work[i - 1][0] != pending_norm[0][0]):
                    emit_norm()

        # ---------------- Phase C: output projection ------------------------
        # two [128,512] outputs per [128,1024] psum tile
        with tc.tile_pool(name="ysbp", bufs=3) as ysbp:
            for n in range(NO):
                for sq2 in range(ST // 2):
                    yp = ptile(("P0", "P3", "P1", "P2")[sq2 % 4])
                    for half in range(2):
                        sq = 2 * sq2 + half
                        for k in range(NHQ):
                            nc.tensor.matmul(
                                yp[:, 512 * half:512 * (half + 1)],
                                outT[k][:, 128 * sq:128 * (sq + 1)],
                                wo_sb[:, k, 512 * n:512 * (n + 1)],
                                start=(k == 0), stop=(k == NHQ - 1),
                                skip_group_check=True)
                    for half in range(2):
                        sq = 2 * sq2 + half
                        ys = ysbp.tile([128, 512], BF16, tag=f"ys{half}",
                                       name=f"ys{half}")
                        if (sq2 + half) % 2 == 0:
                            nc.vector.tensor_copy(
                                ys[:], yp[:, 512 * half:512 * (half + 1)])
                        else:
                            nc.scalar.copy(
                                ys[:], yp[:, 512 * half:512 * (half + 1)])
                        nc.sync.dma_start(
                            out=y[128 * sq:128 * (sq + 1),
                                  512 * n:512 * (n + 1)],
                            in_=ys[:])

        if debug_taps:
            for h in range(NHQ):
                nc.sync.dma_start(out=qt_d[h], in_=qt[h][:])
                nc.sync.dma_start(out=o_d[h], in_=outT[h][:])
            nc.sync.dma_start(out=kt_d[:], in_=kt[:])
            nc.sync.dma_start(out=v_d[:], in_=v_sb[:])

    nc.compile()
    return nc


def get_nc(generic_mask=False):
    key = "ncg" if generic_mask else "nc"
    if key not in _NC_CACHE:
        _NC_CACHE[key] = build_nc(generic_mask)
    return _NC_CACHE[key]


def _is_causal(mask):
    m2 = np.asarray(mask, dtype=np.float32)[0, 0]
    if m2.shape != (S, S):
        return False
    # quick grid check, then full check
    idx = np.arange(0, S, 97)
    sub = m2[np.ix_(idx, idx)]
    expect = np.where(idx[None, :] > idx[:, None], np.float32(-1e9), 0.0)
    if not np.array_equal(sub, expect):
        return False
    full = np.triu(np.full((S, S), np.float32(-1e9)), 1)
    return np.array_equal(m2, full)


def make_in_maps(hidden_states, attention_mask, position_ids, Wq, Wk, Wv, Wo,
                 generic_mask=False):
    hs = np.asarray(hidden_states, dtype=np.float32)
    pos = np.asarray(position_ids)
    Wq = np.asarray(Wq, dtype=np.float32)
    Wk = np.asarray(Wk, dtype=np.float32)
    Wv = np.asarray(Wv, dtype=np.float32)
    Wo = np.asarray(Wo, dtype=np.float32)
    assert hs.shape == (1, S, HID)
    assert Wq.shape == (HID, HID) and Wk.shape == (HID, 1024)
    assert Wv.shape == (HID, 1024) and Wo.shape == (HID, HID)

    xT = np.ascontiguousarray(hs[0].T).astype(NPBF16)

    p = pos[0].astype(np.float32)
    inv = (1.0 / (10000.0 ** (np.arange(0, D, 2, dtype=np.float32)
                              / np.float32(D)))).astype(np.float32)
    freqs = p[:, None] * inv[None, :]
    emb = np.concatenate([freqs, freqs], axis=1)        # (S, 128)
    cosT = np.ascontiguousarray(np.cos(emb).T).astype(NPBF16)
    sinT = np.sin(emb).T.astype(np.float32)
    sinT[:64] *= np.float32(-1.0)
    # roll by 64 partitions: row p holds sign-folded sin[(p+64)%128]
    sinT = np.roll(sinT, 64, axis=0)
    sinTf = np.ascontiguousarray(sinT).astype(NPBF16)
    ones = np.ones((128, 1), dtype=NPBF16)

    if generic_mask:
        m2 = np.asarray(attention_mask, dtype=np.float32)[0, 0]
        maskT = np.stack([
            np.ascontiguousarray(
                m2[512 * (t // 4):512 * (t // 4 + 1),
                   128 * t:128 * (t + 1)].T) / np.float32(SCALE)
            for t in range(ST)
        ]).astype(NPBF16)

    in_maps = []
    for c in range(NCORES):
        wq_c = Wq[:, 512 * c:512 * (c + 1)]              # [4096, 512]
        wk_c = Wk[:, 128 * c:128 * (c + 1)]              # [4096, 128]
        wv_c = Wv[:, 128 * c:128 * (c + 1)]
        wo_c = Wo[512 * c:512 * (c + 1), :]              # [512, 4096]
        # wq_pre[g][p, kk, f] = wq_c[512g + 128kk + p, f]
        wq_pre = np.ascontiguousarray(
            wq_c.reshape(8, 4, 128, 512).transpose(0, 2, 1, 3)).astype(NPBF16)
        wkv_c = np.concatenate([wk_c, wv_c], axis=1)     # [4096, 256]
        wkv_pre = np.ascontiguousarray(
            wkv_c.reshape(8, 4, 128, 256).transpose(0, 2, 1, 3)).astype(NPBF16)
        # wo_sb[p, kh, o] = wo_c[128kh + p, o]
        wo_pre = np.ascontiguousarray(
            wo_c.reshape(4, 128, HID).transpose(1, 0, 2)).astype(NPBF16)
        m = {
            "xT": xT,
            "wq_pre": wq_pre,
            "wkv_pre": wkv_pre,
            "wo_pre": wo_pre,
            "cosT": cosT,
            "sinTf": sinTf,
            "ones": ones,
        }
        if generic_mask:
            m["maskT"] = maskT
        in_maps.append(m)
    return in_maps


def kernel(hidden_states, attention_mask, position_ids, Wq, Wk, Wv, Wo):
    os.environ["BASS_NEVER_TRACE"] = "1"
    generic = not _is_causal(attention_mask)
    in_maps = make_in_maps(hidden_states, attention_mask, position_ids,
                           Wq, Wk, Wv, Wo, generic_mask=generic)
    nc = get_nc(generic_mask=generic)
    res = run_bass_kernel_spmd(nc, in_maps, list(range(NCORES)))
    acc = np.zeros((S, HID), dtype=np.float64)
    for c in range(NCORES):
        acc += res.results[c]["y"]
    return acc.astype(np.float32)[None]


# revision 5
# speedup vs baseline: 1.0051x; 1.0051x over previous
"""Grouped-query attention (B=1, S=2048, HID=4096, 32 q-heads / 8 kv-heads,
D=128, RoPE, additive causal mask) on 8 Trainium2 NeuronCores.

Sharding (tensor-parallel over heads, per the sharding hint): core c owns 4
q-heads (columns 512c:512c+512 of Wq), kv-head c (columns 128c:128c+128 of
Wk/Wv), and rows 512c:512c+512 of Wo. Each core emits a full-shape partial
of the output projection; the host sums the 8 partials (the "all-reduce" of
the row-sharded Wo matmul).

Design notes (TimelineSim ~388us/core vs 606us baseline):
  - All matmuls bf16 (1 PE cycle/row like f32r, half the DMA/SBUF; fp8 was
    measured numerically unacceptable for the 2e-2 gate). Weights are
    host-prepacked into their exact SBUF layouts so every DMA moves large
    contiguous descriptors.
  - One unified PSUM scheme: four [128,1024] double-bank tiles (P0..P3)
    tag-rotated through warmup / QKV+RoPE / attention / output projection.
    No psum pool open/close between phases, so there are no cross-phase
    allocation barriers - a new use of a tag only waits for that tag's
    previous consumers.
  - PE warmup chain at t~0 (tiny matmuls on a zeroed tile) so the tensor
    engine p-state ramp (0.65/1.2/2.4 GHz) is complete and the PE never
    idles while the first weight/x DMAs land.
  - Causal masking is free-form: score tiles are computed at variable width
    (only columns right of the diagonal), exp'd unmasked, and the single
    boundary 128-block is multiplied by a constant lower-triangular 0/1
    bf16 mask on the DVE (2x mode). No mask DMA, no PSUM mask adds.
    A generic additive-mask fallback is built if the host detects a
    non-causal attention_mask input.
  - Attention is one software-pipelined stream of k-tile PAIRS: two k-tiles
    share a [128,1024] psum score tile and (off-diagonal) a single exp, so
    the Act engine's fixed per-op overhead is halved; lsum(ones matmul) and
    PV accumulate per tile with interleaved accumulation groups.
  - V is produced directly in [s, d] layout (x chunk as the stationary
    operand) - no PE transposes. Q/K run feature-major with RoPE applied by
    the DVE in 2x bf16 mode from an Act-drained copy of the psum (the
    sign-folded sin table is pre-rolled 64 partitions so both SBUF operands
    of each half-multiply share a base partition, a BIR requirement).
  - Softmax normalization: DVE reciprocal of the lsum row, Pool
    partition_broadcast, one DVE multiply into the bf16 outT tile.
  - Output projection drains psum via two independent [128,512] copies
    (alternating Act/DVE) so each half's y DMA fires as soon as its copy
    lands; y is written bf16 and the partials are summed f32 on the host.
"""
import os

import numpy as np
import ml_dtypes
from contextlib import ExitStack

import concourse.bass as bass
import concourse.tile as tile
from concourse import bacc, mybir
from concourse.bass_utils import run_bass_kernel_spmd

F32 = mybir.dt.float32
BF16 = mybir.dt.bfloat16
EXP = mybir.ActivationFunctionType.Exp
NPBF16 = ml_dtypes.bfloat16

S = 2048
HID = 4096
D = 128
NCORES = 8
NHQ = 4                      # q heads per core
SCALE = float(D) ** -0.5
ST = S // 128                # 16 s-tiles
SL = S // 512                # 4 s-slices
KT = HID // 128              # 32 hidden k-tiles
NO = HID // 512              # 8 output column slices
NWARM = 90                   # PE warmup matmuls (bridges first DMAs)

_NC_CACHE = {}


def build_nc(generic_mask=False, nwarm=None, debug_taps=False):
    nc = bacc.Bacc("TRN2", target_bir_lowering=False, debug=False,
                   num_devices=NCORES)
    xT = nc.dram_tensor("xT", [HID, S], BF16, kind="ExternalInput").ap()
    wq_pre = nc.dram_tensor("wq_pre", [8, 128, 4, 512], BF16,
                            kind="ExternalInput").ap()
    wkv_pre = nc.dram_tensor("wkv_pre", [8, 128, 4, 256], BF16,
                             kind="ExternalInput").ap()
    wo_pre = nc.dram_tensor("wo_pre", [128, NHQ, HID], BF16,
                            kind="ExternalInput").ap()
    cosT = nc.dram_tensor("cosT", [128, S], BF16, kind="ExternalInput").ap()
    sinTf = nc.dram_tensor("sinTf", [128, S], BF16, kind="ExternalInput").ap()
    ones = nc.dram_tensor("ones", [128, 1], BF16, kind="ExternalInput").ap()
    if generic_mask:
        maskT = nc.dram_tensor("maskT", [ST, 128, 512], BF16,
                               kind="ExternalInput").ap()
    y = nc.dram_tensor("y", [S, HID], BF16, kind="ExternalOutput").ap()
    if debug_taps:
        qt_d = nc.dram_tensor("qt_d", [NHQ, 128, S], BF16,
                              kind="ExternalOutput").ap()
        kt_d = nc.dram_tensor("kt_d", [128, S], BF16,
                              kind="ExternalOutput").ap()
        v_d = nc.dram_tensor("v_d", [128, ST, 128], BF16,
                             kind="ExternalOutput").ap()
        o_d = nc.dram_tensor("o_d", [NHQ, 128, S], BF16,
                             kind="ExternalOutput").ap()

    with tile.TileContext(nc) as tc, ExitStack() as ctx:
        const = ctx.enter_context(tc.tile_pool(name="const", bufs=1))
        cos_sb = const.tile([128, S], BF16)
        sin_sb = const.tile([128, S], BF16)
        ones_sb = const.tile([128, 1], BF16)
        qt = [const.tile([128, S], BF16, tag=f"qt{h}", name=f"qt{h}")
              for h in range(NHQ)]
        kt = const.tile([128, S], BF16)
        v_sb = const.tile([128, ST, 128], BF16)
        outT = [const.tile([128, S], BF16, tag=f"outT{h}", name=f"outT{h}")
                for h in range(NHQ)]
        tri = const.tile([128, 128], BF16)
        if generic_mask:
            mask_sb = const.tile([128, ST, 512], BF16)

        # Unified PSUM: four 2-bank [128,1024] tiles, tag-rotated through all
        # phases. No pool open/close -> no cross-phase handover barriers; a
        # new instance of tag Pn only waits for the previous Pn consumers.
        psum = ctx.enter_context(tc.tile_pool(name="psum", bufs=1,
                                              space="PSUM"))

        def ptile(tag):
            return psum.tile([128, 1024], F32, tag=tag, name=tag)

        # ---------------- PE warmup: own the clock ramp from t~0 ------------
        warm = ctx.enter_context(tc.tile_pool(name="warm", bufs=1))
        wsb = warm.tile([128, 64], BF16)
        nc.gpsimd.memset(wsb[:], 0.0)
        wps = ptile("P3")
        for _ in range(nwarm if nwarm is not None else NWARM):
            nc.tensor.matmul(wps[0:64, 0:64], wsb[:, 0:64], wsb[:, 0:64],
                             start=True, stop=True, skip_group_check=True)
        nc.gpsimd.memset(tri[:], 1.0)
        nc.gpsimd.affine_select(out=tri[:], in_=tri[:],
                                compare_op=mybir.AluOpType.is_ge, fill=0.0,
                                base=0, pattern=[[1, 128]],
                                channel_multiplier=-1)

        # ---------------- Phase A: projections + RoPE -----------------------
        wA = ctx.enter_context(tc.tile_pool(name="wA", bufs=1))
        wq_sb = wA.tile([128, KT, 512], BF16)
        wkv_sb = wA.tile([128, KT, 256], BF16)
        wo_sb = wA.tile([128, NHQ, HID], BF16)

        with tc.tile_pool(name="xtp", bufs=4) as xtp, \
             tc.tile_pool(name="xtp2", bufs=4) as xtp2, \
             tc.tile_pool(name="drains", bufs=2) as drains, \
             tc.tile_pool(name="ropes", bufs=2) as ropes:

            def rope_from(dst_slice, sb, cs, sn):
                """dst = sb*cos + rotate_half(sb)*sin; all-bf16 -> DVE 2x.

                sn is the sign-folded sin table rolled by 64 partitions, so
                each half-multiply reads both SBUF operands at the SAME base
                partition (BIR requires equal input bases for SB+SB ops).
                """
                rot = ropes.tile([128, 512], BF16, tag="rot")
                nc.vector.tensor_mul(rot[0:64, :], sb[64:128, :],
                                     sn[64:128, :])
                nc.vector.tensor_mul(rot[64:128, :], sb[0:64, :], sn[0:64, :])
                nc.vector.tensor_mul(dst_slice, sb[:, :], cs)
                nc.vector.tensor_add(dst_slice, dst_slice, rot[:])

            for j in range(SL):
                # P0 = [q0|q1], P1 = [q2|q3], P2 = [k|v]
                acc = [ptile("P0"), ptile("P1"), ptile("P2")]
                qps = [acc[0][:, 0:512], acc[0][:, 512:1024],
                       acc[1][:, 0:512], acc[1][:, 512:1024]]
                kps = acc[2][:, 0:512]
                vtile = acc[2][:, 512:1024]
                for g in range(8):
                    if j == 0:
                        # weights ahead of x in queue order; g=0 in halves so
                        # the first matmul group can fire earlier
                        xt = xtp.tile([128, 4, 512], BF16, tag="xt")
                        xts = xt
                        for hh in ([0, 1] if g == 0 else [None]):
                            if hh is None:
                                ksl = slice(4 * g, 4 * g + 4)
                                kk2 = slice(0, 4)
                            else:
                                ksl = slice(2 * hh, 2 * hh + 2)
                                kk2 = slice(2 * hh, 2 * hh + 2)
                            nc.sync.dma_start(out=wq_sb[:, ksl, :],
                                              in_=wq_pre[g][:, kk2, :])
                            nc.sync.dma_start(out=wkv_sb[:, ksl, :],
                                              in_=wkv_pre[g][:, kk2, :])
                            nc.sync.dma_start(
                                out=xt[:, kk2, :],
                                in_=xT[512 * g + 256 * (hh or 0):
                                       512 * g + 256 * (hh or 0) +
                                       (512 if hh is None else 256),
                                       512 * j:512 * (j + 1)]
                                .rearrange("(t p) m -> p t m", p=128))
                        if g == 7:
                            nc.sync.dma_start(out=cos_sb[:], in_=cosT[:])
                            nc.sync.dma_start(out=sin_sb[:], in_=sinTf[:])
                            nc.sync.dma_start(out=ones_sb[:], in_=ones[:])
                            if generic_mask:
                                nc.sync.dma_start(
                                    out=mask_sb[:],
                                    in_=maskT.rearrange("t p q -> p t q"))
                    else:
                        if g % 2 == 0:
                            xt = xtp2.tile([128, 8, 512], BF16, tag="xt2")
                            nc.sync.dma_start(
                                out=xt[:],
                                in_=xT[1024 * (g // 2):1024 * (g // 2 + 1),
                                       512 * j:512 * (j + 1)]
                                .rearrange("(t p) m -> p t m", p=128))
                        xts = xt[:, 4 * (g % 2):4 * (g % 2 + 1), :]
                    if j == 2 and g < 4:
                        nc.sync.dma_start(out=wo_sb[:, g, :],
                                          in_=wo_pre[:, g, :])
                    def q_mm(kk, f):
                        k = 4 * g + kk
                        nc.tensor.matmul(
                            qps[f], wq_sb[:, k, 128 * f:128 * (f + 1)],
                            xts[:, kk, :], start=(k == 0), stop=(k == KT - 1),
                            skip_group_check=True)

                    def kv_mm(kk):
                        k = 4 * g + kk
                        st, sp = (k == 0), (k == KT - 1)
                        nc.tensor.matmul(kps, wkv_sb[:, k, 0:128],
                                         xts[:, kk, :], start=st, stop=sp,
                                         skip_group_check=True)
                        # V in [s, d] layout: x chunk is the stationary
                        # side. start only on sb_==0: PSUM start zeroing is
                        # 2KB-region wide, so one start covers the whole
                        # vtile bank; per-sb_ starts would wipe each other's
                        # k=0 contribution.
                        for sb_ in range(4):
                            nc.tensor.matmul(
                                vtile[:, 128 * sb_:128 * (sb_ + 1)],
                                xts[:, kk, 128 * sb_:128 * (sb_ + 1)],
                                wkv_sb[:, k, 128:256],
                                start=st and sb_ == 0, stop=sp,
                                skip_group_check=True)

                    if g < 7:
                        for kk in range(4):
                            for f in range(NHQ):
                                q_mm(kk, f)
                        for kk in range(4):
                            kv_mm(kk)
                    elif True:
                        # last chunk: q features in drain order first, so
                        # the paired Act drains start ~4us before the j
                        # boundary and the next j's banks are free in time
                        for f in range(NHQ):
                            for kk in range(4):
                                q_mm(kk, f)
                        for kk in range(4):
                            kv_mm(kk)
                cs = cos_sb[:, 512 * j:512 * (j + 1)]
                sn = sin_sb[:, 512 * j:512 * (j + 1)]
                # paired Act drains free each 2-bank tile with one copy
                dr0 = drains.tile([128, 1024], BF16, tag="dr0")
                nc.scalar.copy(dr0[:], acc[0][:])
                dr1 = drains.tile([128, 1024], BF16, tag="dr1")
                nc.scalar.copy(dr1[:], acc[1][:])
                drk = drains.tile([128, 512], BF16, tag="drk")
                nc.scalar.copy(drk[:], kps)
                nc.scalar.copy(v_sb[:, 4 * j:4 * (j + 1), :], vtile[:, :])
                rope_from(qt[0][:, 512 * j:512 * (j + 1)], dr0[:, 0:512],
                          cs, sn)
                rope_from(qt[1][:, 512 * j:512 * (j + 1)], dr0[:, 512:1024],
                          cs, sn)
                rope_from(qt[2][:, 512 * j:512 * (j + 1)], dr1[:, 0:512],
                          cs, sn)
                rope_from(qt[3][:, 512 * j:512 * (j + 1)], dr1[:, 512:1024],
                          cs, sn)
                rope_from(kt[:, 512 * j:512 * (j + 1)], drk, cs, sn)

        # ---------------- Phase B: attention --------------------------------
        # One continuous, globally software-pipelined stream of score pairs
        # across all (h, j): sc of pair i+1 overlaps exp/select of pair i,
        # including across (h, j) boundaries, so Act latency never starves PE.
        # scp pairs rotate P3/P0; the [pv | lsum] tile alternates P1/P2.
        with tc.tile_pool(name="ptbp", bufs=4) as ptbp, \
             tc.tile_pool(name="rbcp", bufs=2) as rbcp:
            work = []          # (h, j, ta, tb) in stream order
            for h in range(NHQ):
                for j in range(SL):
                    for p in range((4 * j + 4) // 2):
                        work.append((h, j, 2 * p, 2 * p + 1))

            po = {}
            pts = {}

            def emit_sc(i):
                h, j, ta, tb = work[i]
                if (h, j) not in po:
                    po[(h, j)] = ptile("P1" if (4 * h + j) % 2 == 0 else "P2")

                def off(t):
                    return max(0, 128 * (t - 4 * j))

                oa, ob = off(ta), off(tb)
                scp = ptile("P3" if i % 2 == 0 else "P0")
                nc.tensor.matmul(
                    scp[:, oa:512], kt[:, 128 * ta:128 * (ta + 1)],
                    qt[h][:, 512 * j + oa:512 * (j + 1)],
                    start=True, stop=True, skip_group_check=True)
                nc.tensor.matmul(
                    scp[:, 512 + ob:1024], kt[:, 128 * tb:128 * (tb + 1)],
                    qt[h][:, 512 * j + ob:512 * (j + 1)],
                    start=True, stop=True, skip_group_check=True)
                if generic_mask and tb >= 4 * j:
                    if ta >= 4 * j:
                        nc.vector.tensor_add(scp[:, oa:512], scp[:, oa:512],
                                             mask_sb[:, ta, oa:])
                    nc.vector.tensor_add(scp[:, 512 + ob:1024],
                                         scp[:, 512 + ob:1024],
                                         mask_sb[:, tb, ob:])
                if ta == 0:
                    # first pair of an (h, j): separate tiles so ls/pv of ta
                    # only waits one 512-wide exp, not the whole pair
                    pta = ptbp.tile([128, 512], BF16, tag="ptbf",
                                    name="pta")
                    ptb2 = ptbp.tile([128, 512], BF16, tag="ptbf",
                                     name="ptb2")
                    nc.scalar.activation(pta[:, oa:], scp[:, oa:512],
                                         EXP, bias=0.0, scale=SCALE)
                    nc.scalar.activation(ptb2[:, ob:], scp[:, 512 + ob:1024],
                                         EXP, bias=0.0, scale=SCALE)
                    if not generic_mask:
                        if ta >= 4 * j:
                            nc.vector.tensor_mul(pta[:, oa:oa + 128],
                                                 pta[:, oa:oa + 128], tri[:])
                        if tb >= 4 * j:
                            nc.vector.tensor_mul(ptb2[:, ob:ob + 128],
                                                 ptb2[:, ob:ob + 128],
                                                 tri[:])
                    pts[(h, j, ta)] = pta[:, oa:]
                    pts[(h, j, tb)] = ptb2[:, ob:]
                    return
                ptb = ptbp.tile([128, 1024], BF16, tag="ptb")
                if ta >= 4 * j:          # diagonal pair: two exps
                    nc.scalar.activation(ptb[:, oa:512], scp[:, oa:512],
                                         EXP, bias=0.0, scale=SCALE)
                    nc.scalar.activation(ptb[:, 512 + ob:1024],
                                         scp[:, 512 + ob:1024], EXP,
                                         bias=0.0, scale=SCALE)
                else:                    # one exp across the pair
                    nc.scalar.activation(ptb[:, oa:1024], scp[:, oa:1024],
                                         EXP, bias=0.0, scale=SCALE)
                if not generic_mask:
                    for tx, ox, base in ((ta, oa, 0), (tb, ob, 512)):
                        if tx >= 4 * j:
                            # staircase confined to the first valid block:
                            # multiply by the const lower-tri mask (DVE 2x)
                            nc.vector.tensor_mul(
                                ptb[:, base + ox:base + ox + 128],
                                ptb[:, base + ox:base + ox + 128], tri[:])
                pts[(h, j, ta)] = ptb[:, oa:512]
                pts[(h, j, tb)] = ptb[:, 512 + ob:1024]

            def emit_lspv(i):
                h, j, ta, tb = work[i]
                tmax = 4 * j + 4
                p = po[(h, j)]
                ops, lps = p[:, 0:512], p[0:1, 512:1024]
                for u in (ta, tb):
                    o = max(0, 128 * (u - 4 * j))
                    pu = pts.pop((h, j, u))
                    nc.tensor.matmul(
                        lps[:, o:512], ones_sb[:], pu,
                        start=(u == 0), stop=(u == tmax - 1),
                        skip_group_check=True)
                    nc.tensor.matmul(
                        ops[:, o:512], v_sb[:, u, :], pu,
                        start=(u == 0), stop=(u == tmax - 1),
                        skip_group_check=True)
                if tb == tmax - 1:       # (h, j) complete
                    pending_norm.append((h, j))

            def emit_norm():
                h, j = pending_norm.pop(0)
                p = po[(h, j)]
                ops, lps = p[:, 0:512], p[0:1, 512:1024]
                # stage 1/l into SBUF (DVE reciprocal can read PSUM; GPSIMD
                # cannot), broadcast on Pool, one DVE multiply
                rsb = rbcp.tile([1, 512], F32, tag="rsb")
                nc.vector.reciprocal(rsb[:], lps[:, 0:512])
                rb = rbcp.tile([128, 512], F32, tag="rb")
                nc.gpsimd.partition_broadcast(rb[:], rsb[:])
                nc.vector.tensor_mul(
                    outT[h][:, 512 * j:512 * (j + 1)], ops, rb[:])
                del po[(h, j)]

            pending_norm = []
            for i in range(len(work) + 2):
                if i < len(work):
                    emit_sc(i)
                if 1 <= i <= len(work):
                    emit_lspv(i - 1)
                # normalize deferred one step so its DVE ops rank behind the
                # next iteration's critical tri-mask multiplies
                if pending_norm and (i >= len(work) or
                                     work[i - 1][1] != pending_norm[0][1] or
                                     work[i - 1][0] != pending_norm[0][0]):
                    emit_norm()

        # ---------------- Phase C: output projection ------------------------
        # two [128,512] outputs per [128,1024] psum tile
        with tc.tile_pool(name="ysbp", bufs=3) as ysbp:
            for n in range(NO):
                for sq2 in range(ST // 2):
                    yp = ptile(("P0", "P3", "P1", "P2")[sq2 % 4])
                    for half in range(2):
                        sq = 2 * sq2 + half
                        for k in range(NHQ):
                            nc.tensor.matmul(
                                yp[:, 512 * half:512 * (half + 1)],
                                outT[k][:, 128 * sq:128 * (sq + 1)],
                                wo_sb[:, k, 512 * n:512 * (n + 1)],
                                start=(k == 0), stop=(k == NHQ - 1),
                                skip_group_check=True)
                    for half in range(2):
                        sq = 2 * sq2 + half
                        ys = ysbp.tile([128, 512], BF16, tag=f"ys{half}",
                                       name=f"ys{half}")
                        if (sq2 + half) % 2 == 0:
                            nc.vector.tensor_copy(
                                ys[:], yp[:, 512 * half:512 * (half + 1)])
                        else:
                            nc.scalar.copy(
                                ys[:], yp[:, 512 * half:512 * (half + 1)])
                        nc.sync.dma_start(
                            out=y[128 * sq:128 * (sq + 1),
                                  512 * n:512 * (n + 1)],
                            in_=ys[:])

        if debug_taps:
            for h in range(NHQ):
                nc.sync.dma_start(out=qt_d[h], in_=qt[h][:])
                nc.sync.dma_start(out=o_d[h], in_=outT[h][:])
            nc.sync.dma_start(out=kt_d[:], in_=kt[:])
            nc.sync.dma_start(out=v_d[:], in_=v_sb[:])

    nc.compile()
    return nc


def get_nc(generic_mask=False):
    key = "ncg" if generic_mask else "nc"
    if key not in _NC_CACHE:
        _NC_CACHE[key] = build_nc(generic_mask)
    return _NC_CACHE[key]


def _is_causal(mask):
    m2 = np.asarray(mask, dtype=np.float32)[0, 0]
    if m2.shape != (S, S):
        return False
    # quick grid check, then full check
    idx = np.arange(0, S, 97)
    sub = m2[np.ix_(idx, idx)]
    expect = np.where(idx[None, :] > idx[:, None], np.float32(-1e9), 0.0)
    if not np.array_equal(sub, expect):
        return False
    full = np.triu(np.full((S, S), np.float32(-1e9)), 1)
    return np.array_equal(m2, full)


def make_in_maps(hidden_states, attention_mask, position_ids, Wq, Wk, Wv, Wo,
                 generic_mask=False):
    hs = np.asarray(hidden_states, dtype=np.float32)
    pos = np.asarray(position_ids)
    Wq = np.asarray(Wq, dtype=np.float32)
    Wk = np.asarray(Wk, dtype=np.float32)
    Wv = np.asarray(Wv, dtype=np.float32)
    Wo = np.asarray(Wo, dtype=np.float32)
    assert hs.shape == (1, S, HID)
    assert Wq.shape == (HID, HID) and Wk.shape == (HID, 1024)
    assert Wv.shape == (HID, 1024) and Wo.shape == (HID, HID)

    xT = np.ascontiguousarray(hs[0].T).astype(NPBF16)

    p = pos[0].astype(np.float32)
    inv = (1.0 / (10000.0 ** (np.arange(0, D, 2, dtype=np.float32)
                              / np.float32(D)))).astype(np.float32)
    freqs = p[:, None] * inv[None, :]
    emb = np.concatenate([freqs, freqs], axis=1)        # (S, 128)
    cosT = np.ascontiguousarray(np.cos(emb).T).astype(NPBF16)
    sinT = np.sin(emb).T.astype(np.float32)
    sinT[:64] *= np.float32(-1.0)
    # roll by 64 partitions: row p holds sign-folded sin[(p+64)%128]
    sinT = np.roll(sinT, 64, axis=0)
    sinTf = np.ascontiguousarray(sinT).astype(NPBF16)
    ones = np.ones((128, 1), dtype=NPBF16)

    if generic_mask:
        m2 = np.asarray(attention_mask, dtype=np.float32)[0, 0]
        maskT = np.stack([
            np.ascontiguousarray(
                m2[512 * (t // 4):512 * (t // 4 + 1),
                   128 * t:128 * (t + 1)].T) / np.float32(SCALE)
            for t in range(ST)
        ]).astype(NPBF16)

    in_maps = []
    for c in range(NCORES):
        wq_c = Wq[:, 512 * c:512 * (c + 1)]              # [4096, 512]
        wk_c = Wk[:, 128 * c:128 * (c + 1)]              # [4096, 128]
        wv_c = Wv[:, 128 * c:128 * (c + 1)]
        wo_c = Wo[512 * c:512 * (c + 1), :]              # [512, 4096]
        # wq_pre[g][p, kk, f] = wq_c[512g + 128kk + p, f]
        wq_pre = np.ascontiguousarray(
            wq_c.reshape(8, 4, 128, 512).transpose(0, 2, 1, 3)).astype(NPBF16)
        wkv_c = np.concatenate([wk_c, wv_c], axis=1)     # [4096, 256]
        wkv_pre = np.ascontiguousarray(
            wkv_c.reshape(8, 4, 128, 256).transpose(0, 2, 1, 3)).astype(NPBF16)
        # wo_sb[p, kh, o] = wo_c[128kh + p, o]
        wo_pre = np.ascontiguousarray(
            wo_c.reshape(4, 128, HID).transpose(1, 0, 2)).astype(NPBF16)
        m = {
            "xT": xT,
            "wq_pre": wq_pre,
            "wkv_pre": wkv_pre,
            "wo_pre": wo_pre,
            "cosT": cosT,
            "sinTf": sinTf,
            "ones": ones,
        }
        if generic_mask:
            m["maskT"] = maskT
        in_maps.append(m)
    return in_maps


def kernel(hidden_states, attention_mask, position_ids, Wq, Wk, Wv, Wo):
    os.environ["BASS_NEVER_TRACE"] = "1"
    generic = not _is_causal(attention_mask)
    in_maps = make_in_maps(hidden_states, attention_mask, position_ids,
                           Wq, Wk, Wv, Wo, generic_mask=generic)
    nc = get_nc(generic_mask=generic)
    res = run_bass_kernel_spmd(nc, in_maps, list(range(NCORES)))
    acc = np.zeros((S, HID), dtype=np.float64)
    for c in range(NCORES):
        acc += res.results[c]["y"]
    return acc.astype(np.float32)[None]


# revision 6
# speedup vs baseline: 1.0063x; 1.0012x over previous
"""Grouped-query attention (B=1, S=2048, HID=4096, 32 q-heads / 8 kv-heads,
D=128, RoPE, additive causal mask) on 8 Trainium2 NeuronCores.

Sharding (tensor-parallel over heads, per the sharding hint): core c owns 4
q-heads (columns 512c:512c+512 of Wq), kv-head c (columns 128c:128c+128 of
Wk/Wv), and rows 512c:512c+512 of Wo. Each core emits a full-shape partial
of the output projection; the host sums the 8 partials (the "all-reduce" of
the row-sharded Wo matmul).

Design notes (TimelineSim ~388us/core vs 606us baseline):
  - All matmuls bf16 (1 PE cycle/row like f32r, half the DMA/SBUF; fp8 was
    measured numerically unacceptable for the 2e-2 gate). Weights are
    host-prepacked into their exact SBUF layouts so every DMA moves large
    contiguous descriptors.
  - One unified PSUM scheme: four [128,1024] double-bank tiles (P0..P3)
    tag-rotated through warmup / QKV+RoPE / attention / output projection.
    No psum pool open/close between phases, so there are no cross-phase
    allocation barriers - a new use of a tag only waits for that tag's
    previous consumers.
  - PE warmup chain at t~0 (tiny matmuls on a zeroed tile) so the tensor
    engine p-state ramp (0.65/1.2/2.4 GHz) is complete and the PE never
    idles while the first weight/x DMAs land.
  - Causal masking is free-form: score tiles are computed at variable width
    (only columns right of the diagonal), exp'd unmasked, and the single
    boundary 128-block is multiplied by a constant lower-triangular 0/1
    bf16 mask on the DVE (2x mode). No mask DMA, no PSUM mask adds.
    A generic additive-mask fallback is built if the host detects a
    non-causal attention_mask input.
  - Attention is one software-pipelined stream of k-tile PAIRS: two k-tiles
    share a [128,1024] psum score tile and (off-diagonal) a single exp, so
    the Act engine's fixed per-op overhead is halved; lsum(ones matmul) and
    PV accumulate per tile with interleaved accumulation groups.
  - V is produced directly in [s, d] layout (x chunk as the stationary
    operand) - no PE transposes. Q/K run feature-major with RoPE applied by
    the DVE in 2x bf16 mode from an Act-drained copy of the psum (the
    sign-folded sin table is pre-rolled 64 partitions so both SBUF operands
    of each half-multiply share a base partition, a BIR requirement).
  - Softmax normalization: DVE reciprocal of the lsum row, Pool
    partition_broadcast, one DVE multiply into the bf16 outT tile.
  - Output projection drains psum via two independent [128,512] copies
    (alternating Act/DVE) so each half's y DMA fires as soon as its copy
    lands; y is written bf16 and the partials are summed f32 on the host.
"""
import os

import numpy as np
import ml_dtypes
from contextlib import ExitStack

import concourse.bass as bass
import concourse.tile as tile
from concourse import bacc, mybir
from concourse.bass_utils import run_bass_kernel_spmd

F32 = mybir.dt.float32
BF16 = mybir.dt.bfloat16
EXP = mybir.ActivationFunctionType.Exp
NPBF16 = ml_dtypes.bfloat16

S = 2048
HID = 4096
D = 128
NCORES = 8
NHQ = 4                      # q heads per core
SCALE = float(D) ** -0.5
ST = S // 128                # 16 s-tiles
SL = S // 512                # 4 s-slices
KT = HID // 128              # 32 hidden k-tiles
NO = HID // 512              # 8 output column slices
NWARM = 90                   # PE warmup matmuls (bridges first DMAs)

_NC_CACHE = {}


def build_nc(generic_mask=False, nwarm=None, debug_taps=False):
    nc = bacc.Bacc("TRN2", target_bir_lowering=False, debug=False,
                   num_devices=NCORES)
    xT = nc.dram_tensor("xT", [HID, S], BF16, kind="ExternalInput").ap()
    wq_pre = nc.dram_tensor("wq_pre", [8, 128, 4, 512], BF16,
                            kind="ExternalInput").ap()
    wkv_pre = nc.dram_tensor("wkv_pre", [8, 128, 4, 256], BF16,
                             kind="ExternalInput").ap()
    wo_pre = nc.dram_tensor("wo_pre", [128, NHQ, HID], BF16,
                            kind="ExternalInput").ap()
    cosT = nc.dram_tensor("cosT", [128, S], BF16, kind="ExternalInput").ap()
    sinTf = nc.dram_tensor("sinTf", [128, S], BF16, kind="ExternalInput").ap()
    ones = nc.dram_tensor("ones", [128, 1], BF16, kind="ExternalInput").ap()
    if generic_mask:
        maskT = nc.dram_tensor("maskT", [ST, 128, 512], BF16,
                               kind="ExternalInput").ap()
    y = nc.dram_tensor("y", [S, HID], BF16, kind="ExternalOutput").ap()
    if debug_taps:
        qt_d = nc.dram_tensor("qt_d", [NHQ, 128, S], BF16,
                              kind="ExternalOutput").ap()
        kt_d = nc.dram_tensor("kt_d", [128, S], BF16,
                              kind="ExternalOutput").ap()
        v_d = nc.dram_tensor("v_d", [128, ST, 128], BF16,
                             kind="ExternalOutput").ap()
        o_d = nc.dram_tensor("o_d", [NHQ, 128, S], BF16,
                             kind="ExternalOutput").ap()

    with tile.TileContext(nc) as tc, ExitStack() as ctx:
        const = ctx.enter_context(tc.tile_pool(name="const", bufs=1))
        cos_sb = const.tile([128, S], BF16)
        sin_sb = const.tile([128, S], BF16)
        ones_sb = const.tile([128, 1], BF16)
        qt = [const.tile([128, S], BF16, tag=f"qt{h}", name=f"qt{h}")
              for h in range(NHQ)]
        kt = const.tile([128, S], BF16)
        v_sb = const.tile([128, ST, 128], BF16)
        outT = [const.tile([128, S], BF16, tag=f"outT{h}", name=f"outT{h}")
                for h in range(NHQ)]
        tri = const.tile([128, 128], BF16)
        if generic_mask:
            mask_sb = const.tile([128, ST, 512], BF16)

        # Unified PSUM: four 2-bank [128,1024] tiles, tag-rotated through all
        # phases. No pool open/close -> no cross-phase handover barriers; a
        # new instance of tag Pn only waits for the previous Pn consumers.
        psum = ctx.enter_context(tc.tile_pool(name="psum", bufs=1,
                                              space="PSUM"))

        def ptile(tag):
            return psum.tile([128, 1024], F32, tag=tag, name=tag)

        # ---------------- PE warmup: own the clock ramp from t~0 ------------
        warm = ctx.enter_context(tc.tile_pool(name="warm", bufs=1))
        wsb = warm.tile([128, 64], BF16)
        nc.gpsimd.memset(wsb[:], 0.0)
        wps = ptile("P3")
        for _ in range(nwarm if nwarm is not None else NWARM):
            nc.tensor.matmul(wps[0:64, 0:64], wsb[:, 0:64], wsb[:, 0:64],
                             start=True, stop=True, skip_group_check=True)
        nc.gpsimd.memset(tri[:], 1.0)
        nc.gpsimd.affine_select(out=tri[:], in_=tri[:],
                                compare_op=mybir.AluOpType.is_ge, fill=0.0,
                                base=0, pattern=[[1, 128]],
                                channel_multiplier=-1)

        # ---------------- Phase A: projections + RoPE -----------------------
        wA = ctx.enter_context(tc.tile_pool(name="wA", bufs=1))
        wq_sb = wA.tile([128, KT, 512], BF16)
        wkv_sb = wA.tile([128, KT, 256], BF16)
        wo_sb = wA.tile([128, NHQ, HID], BF16)

        drains = ctx.enter_context(tc.tile_pool(name="drains", bufs=2))
        ropes = ctx.enter_context(tc.tile_pool(name="ropes", bufs=2))
        with tc.tile_pool(name="xtp", bufs=4) as xtp, \
             tc.tile_pool(name="xtp2", bufs=4) as xtp2:

            def rope_from(dst_slice, sb, cs, sn):
                """dst = sb*cos + rotate_half(sb)*sin; all-bf16 -> DVE 2x.

                sn is the sign-folded sin table rolled by 64 partitions, so
                each half-multiply reads both SBUF operands at the SAME base
                partition (BIR requires equal input bases for SB+SB ops).
                """
                rot = ropes.tile([128, 512], BF16, tag="rot")
                nc.vector.tensor_mul(rot[0:64, :], sb[64:128, :],
                                     sn[64:128, :])
                nc.vector.tensor_mul(rot[64:128, :], sb[0:64, :], sn[0:64, :])
                nc.vector.tensor_mul(dst_slice, sb[:, :], cs)
                nc.vector.tensor_add(dst_slice, dst_slice, rot[:])

            for j in range(SL):
                # P0 = [q0|q1], P1 = [q2|q3], P2 = [k|v]
                acc = [ptile("P0"), ptile("P1"), ptile("P2")]
                qps = [acc[0][:, 0:512], acc[0][:, 512:1024],
                       acc[1][:, 0:512], acc[1][:, 512:1024]]
                kps = acc[2][:, 0:512]
                vtile = acc[2][:, 512:1024]
                for g in range(8):
                    if j == 0:
                        # weights ahead of x in queue order; g=0 in halves so
                        # the first matmul group can fire earlier
                        xt = xtp.tile([128, 4, 512], BF16, tag="xt")
                        xts = xt
                        for hh in ([0, 1] if g == 0 else [None]):
                            if hh is None:
                                ksl = slice(4 * g, 4 * g + 4)
                                kk2 = slice(0, 4)
                            else:
                                ksl = slice(2 * hh, 2 * hh + 2)
                                kk2 = slice(2 * hh, 2 * hh + 2)
                            nc.sync.dma_start(out=wq_sb[:, ksl, :],
                                              in_=wq_pre[g][:, kk2, :])
                            nc.sync.dma_start(out=wkv_sb[:, ksl, :],
                                              in_=wkv_pre[g][:, kk2, :])
                            nc.sync.dma_start(
                                out=xt[:, kk2, :],
                                in_=xT[512 * g + 256 * (hh or 0):
                                       512 * g + 256 * (hh or 0) +
                                       (512 if hh is None else 256),
                                       512 * j:512 * (j + 1)]
                                .rearrange("(t p) m -> p t m", p=128))
                        if g == 7:
                            nc.sync.dma_start(out=cos_sb[:], in_=cosT[:])
                            nc.sync.dma_start(out=sin_sb[:], in_=sinTf[:])
                            nc.sync.dma_start(out=ones_sb[:], in_=ones[:])
                            if generic_mask:
                                nc.sync.dma_start(
                                    out=mask_sb[:],
                                    in_=maskT.rearrange("t p q -> p t q"))
                    else:
                        if g % 2 == 0:
                            xt = xtp2.tile([128, 8, 512], BF16, tag="xt2")
                            nc.sync.dma_start(
                                out=xt[:],
                                in_=xT[1024 * (g // 2):1024 * (g // 2 + 1),
                                       512 * j:512 * (j + 1)]
                                .rearrange("(t p) m -> p t m", p=128))
                        xts = xt[:, 4 * (g % 2):4 * (g % 2 + 1), :]
                    if j == 2 and g < 4:
                        nc.sync.dma_start(out=wo_sb[:, g, :],
                                          in_=wo_pre[:, g, :])
                    def q_mm(kk, f):
                        k = 4 * g + kk
                        nc.tensor.matmul(
                            qps[f], wq_sb[:, k, 128 * f:128 * (f + 1)],
                            xts[:, kk, :], start=(k == 0), stop=(k == KT - 1),
                            skip_group_check=True)

                    def kv_mm(kk):
                        k = 4 * g + kk
                        st, sp = (k == 0), (k == KT - 1)
                        nc.tensor.matmul(kps, wkv_sb[:, k, 0:128],
                                         xts[:, kk, :], start=st, stop=sp,
                                         skip_group_check=True)
                        # V in [s, d] layout: x chunk is the stationary
                        # side. start only on sb_==0: PSUM start zeroing is
                        # 2KB-region wide, so one start covers the whole
                        # vtile bank; per-sb_ starts would wipe each other's
                        # k=0 contribution.
                        for sb_ in range(4):
                            nc.tensor.matmul(
                                vtile[:, 128 * sb_:128 * (sb_ + 1)],
                                xts[:, kk, 128 * sb_:128 * (sb_ + 1)],
                                wkv_sb[:, k, 128:256],
                                start=st and sb_ == 0, stop=sp,
                                skip_group_check=True)

                    if g < 7:
                        for kk in range(4):
                            for f in range(NHQ):
                                q_mm(kk, f)
                        for kk in range(4):
                            kv_mm(kk)
                    elif True:
                        # last chunk: q features in drain order first, so
                        # the paired Act drains start ~4us before the j
                        # boundary and the next j's banks are free in time
                        for f in range(NHQ):
                            for kk in range(4):
                                q_mm(kk, f)
                        for kk in range(4):
                            kv_mm(kk)
                cs = cos_sb[:, 512 * j:512 * (j + 1)]
                sn = sin_sb[:, 512 * j:512 * (j + 1)]
                def drain_and_rope(j, acc, kps, vtile, cs, sn):
                    dr0 = drains.tile([128, 1024], BF16, tag="dr0",
                                      name="dr0")
                    dr1 = drains.tile([128, 1024], BF16, tag="dr1",
                                      name="dr1")
                    drk = drains.tile([128, 512], BF16, tag="drk",
                                      name="drk")
                    if j < SL - 1:
                        # paired Act drains free each 2-bank tile in one copy
                        nc.scalar.copy(dr0[:], acc[0][:])
                        nc.scalar.copy(dr1[:], acc[1][:])
                    else:
                        # last slice: 512-wide copies so phase B's first exps
                        # can slot into the Act queue between them
                        nc.scalar.copy(dr0[:, 0:512], acc[0][:, 0:512])
                        nc.scalar.copy(dr0[:, 512:1024], acc[0][:, 512:1024])
                        nc.scalar.copy(dr1[:, 0:512], acc[1][:, 0:512])
                        nc.scalar.copy(dr1[:, 512:1024], acc[1][:, 512:1024])
                    nc.scalar.copy(drk[:], kps)
                    nc.scalar.copy(v_sb[:, 4 * j:4 * (j + 1), :], vtile[:, :])
                    rope_from(qt[0][:, 512 * j:512 * (j + 1)], dr0[:, 0:512],
                              cs, sn)
                    rope_from(qt[1][:, 512 * j:512 * (j + 1)],
                              dr0[:, 512:1024], cs, sn)
                    rope_from(qt[2][:, 512 * j:512 * (j + 1)], dr1[:, 0:512],
                              cs, sn)
                    rope_from(qt[3][:, 512 * j:512 * (j + 1)],
                              dr1[:, 512:1024], cs, sn)
                    rope_from(kt[:, 512 * j:512 * (j + 1)], drk, cs, sn)

                if j < SL - 1:
                    drain_and_rope(j, acc, kps, vtile, cs, sn)
                else:
                    # defer emission of the last slice's drains into the
                    # phase-B stream (see below) so the scheduler ranks B's
                    # entry-critical exps ahead of them
                    deferred_j3 = (lambda j=j, acc=acc, kps=kps,
                                   vtile=vtile, cs=cs, sn=sn:
                                   drain_and_rope(j, acc, kps, vtile,
                                                  cs, sn))

        # ---------------- Phase B: attention --------------------------------
        # One continuous, globally software-pipelined stream of score pairs
        # across all (h, j): sc of pair i+1 overlaps exp/select of pair i,
        # including across (h, j) boundaries, so Act latency never starves PE.
        # scp pairs rotate P3/P0; the [pv | lsum] tile alternates P1/P2.
        with tc.tile_pool(name="ptbp", bufs=4) as ptbp, \
             tc.tile_pool(name="rbcp", bufs=2) as rbcp:
            work = []          # (h, j, ta, tb) in stream order
            for h in range(NHQ):
                for j in range(SL):
                    for p in range((4 * j + 4) // 2):
                        work.append((h, j, 2 * p, 2 * p + 1))

            po = {}
            pts = {}

            def emit_sc(i):
                h, j, ta, tb = work[i]
                if (h, j) not in po:
                    po[(h, j)] = ptile("P1" if (4 * h + j) % 2 == 0 else "P2")

                def off(t):
                    return max(0, 128 * (t - 4 * j))

                oa, ob = off(ta), off(tb)
                scp = ptile("P3" if i % 2 == 0 else "P0")
                nc.tensor.matmul(
                    scp[:, oa:512], kt[:, 128 * ta:128 * (ta + 1)],
                    qt[h][:, 512 * j + oa:512 * (j + 1)],
                    start=True, stop=True, skip_group_check=True)
                nc.tensor.matmul(
                    scp[:, 512 + ob:1024], kt[:, 128 * tb:128 * (tb + 1)],
                    qt[h][:, 512 * j + ob:512 * (j + 1)],
                    start=True, stop=True, skip_group_check=True)
                if generic_mask and tb >= 4 * j:
                    if ta >= 4 * j:
                        nc.vector.tensor_add(scp[:, oa:512], scp[:, oa:512],
                                             mask_sb[:, ta, oa:])
                    nc.vector.tensor_add(scp[:, 512 + ob:1024],
                                         scp[:, 512 + ob:1024],
                                         mask_sb[:, tb, ob:])
                if ta == 0:
                    # first pair of an (h, j): separate tiles so ls/pv of ta
                    # only waits one 512-wide exp, not the whole pair
                    pta = ptbp.tile([128, 512], BF16, tag="ptbf",
                                    name="pta")
                    ptb2 = ptbp.tile([128, 512], BF16, tag="ptbf",
                                     name="ptb2")
                    nc.scalar.activation(pta[:, oa:], scp[:, oa:512],
                                         EXP, bias=0.0, scale=SCALE)
                    nc.scalar.activation(ptb2[:, ob:], scp[:, 512 + ob:1024],
                                         EXP, bias=0.0, scale=SCALE)
                    if not generic_mask:
                        if ta >= 4 * j:
                            nc.vector.tensor_mul(pta[:, oa:oa + 128],
                                                 pta[:, oa:oa + 128], tri[:])
                        if tb >= 4 * j:
                            nc.vector.tensor_mul(ptb2[:, ob:ob + 128],
                                                 ptb2[:, ob:ob + 128],
                                                 tri[:])
                    pts[(h, j, ta)] = pta[:, oa:]
                    pts[(h, j, tb)] = ptb2[:, ob:]
                    return
                ptb = ptbp.tile([128, 1024], BF16, tag="ptb")
                if ta >= 4 * j:          # diagonal pair: two exps
                    nc.scalar.activation(ptb[:, oa:512], scp[:, oa:512],
                                         EXP, bias=0.0, scale=SCALE)
                    nc.scalar.activation(ptb[:, 512 + ob:1024],
                                         scp[:, 512 + ob:1024], EXP,
                                         bias=0.0, scale=SCALE)
                else:                    # one exp across the pair
                    nc.scalar.activation(ptb[:, oa:1024], scp[:, oa:1024],
                                         EXP, bias=0.0, scale=SCALE)
                if not generic_mask:
                    for tx, ox, base in ((ta, oa, 0), (tb, ob, 512)):
                        if tx >= 4 * j:
                            nc.vector.tensor_mul(
                                ptb[:, base + ox:base + ox + 128],
                                ptb[:, base + ox:base + ox + 128], tri[:])
                pts[(h, j, ta)] = ptb[:, oa:512]
                pts[(h, j, tb)] = ptb[:, 512 + ob:1024]

            def emit_lspv(i):
                h, j, ta, tb = work[i]
                tmax = 4 * j + 4
                p = po[(h, j)]
                ops, lps = p[:, 0:512], p[0:1, 512:1024]
                for u in (ta, tb):
                    o = max(0, 128 * (u - 4 * j))
                    pu = pts.pop((h, j, u))
                    nc.tensor.matmul(
                        lps[:, o:512], ones_sb[:], pu,
                        start=(u == 0), stop=(u == tmax - 1),
                        skip_group_check=True)
                    nc.tensor.matmul(
                        ops[:, o:512], v_sb[:, u, :], pu,
                        start=(u == 0), stop=(u == tmax - 1),
                        skip_group_check=True)
                if tb == tmax - 1:       # (h, j) complete
                    pending_norm.append((h, j))

            def emit_norm():
                h, j = pending_norm.pop(0)
                p = po[(h, j)]
                ops, lps = p[:, 0:512], p[0:1, 512:1024]
                # stage 1/l into SBUF (DVE reciprocal can read PSUM; GPSIMD
                # cannot), broadcast on Pool, one DVE multiply
                rsb = rbcp.tile([1, 512], F32, tag="rsb")
                nc.vector.reciprocal(rsb[:], lps[:, 0:512])
                rb = rbcp.tile([128, 512], F32, tag="rb")
                nc.gpsimd.partition_broadcast(rb[:], rsb[:])
                nc.vector.tensor_mul(
                    outT[h][:, 512 * j:512 * (j + 1)], ops, rb[:])
                del po[(h, j)]

            pending_norm = []
            for i in range(len(work) + 2):
                if i == 10:
                    deferred_j3()
                if i < len(work):
                    emit_sc(i)
                if 1 <= i <= len(work):
                    emit_lspv(i - 1)
                # normalize deferred until a second one is pending (or the
                # stream ends) so its DVE ops rank behind the next
                # iteration's critical tri-mask multiplies
                while len(pending_norm) >= 2 or (i >= len(work) + 1 and
                                                 pending_norm):
                    emit_norm()

        # ---------------- Phase C: output projection ------------------------
        # two [128,512] outputs per [128,1024] psum tile
        with tc.tile_pool(name="ysbp", bufs=3) as ysbp:
            for n in range(NO):
                for sq2 in range(ST // 2):
                    yp = ptile(("P0", "P3", "P1", "P2")[sq2 % 4])
                    for half in range(2):
                        sq = 2 * sq2 + half
                        for k in range(NHQ):
                            nc.tensor.matmul(
                                yp[:, 512 * half:512 * (half + 1)],
                                outT[k][:, 128 * sq:128 * (sq + 1)],
                                wo_sb[:, k, 512 * n:512 * (n + 1)],
                                start=(k == 0), stop=(k == NHQ - 1),
                                skip_group_check=True)
                    for half in range(2):
                        sq = 2 * sq2 + half
                        ys = ysbp.tile([128, 512], BF16, tag=f"ys{half}",
                                       name=f"ys{half}")
                        if (sq2 + half) % 2 == 0:
                            nc.vector.tensor_copy(
                                ys[:], yp[:, 512 * half:512 * (half + 1)])
                        else:
                            nc.scalar.copy(
                                ys[:], yp[:, 512 * half:512 * (half + 1)])
                        nc.sync.dma_start(
                            out=y[128 * sq:128 * (sq + 1),
                                  512 * n:512 * (n + 1)],
                            in_=ys[:])

        if debug_taps:
            for h in range(NHQ):
                nc.sync.dma_start(out=qt_d[h], in_=qt[h][:])
                nc.sync.dma_start(out=o_d[h], in_=outT[h][:])
            nc.sync.dma_start(out=kt_d[:], in_=kt[:])
            nc.sync.dma_start(out=v_d[:], in_=v_sb[:])

    nc.compile()
    return nc


def get_nc(generic_mask=False):
    key = "ncg" if generic_mask else "nc"
    if key not in _NC_CACHE:
        _NC_CACHE[key] = build_nc(generic_mask)
    return _NC_CACHE[key]


def _is_causal(mask):
    m2 = np.asarray(mask, dtype=np.float32)[0, 0]
    if m2.shape != (S, S):
        return False
    # quick grid check, then full check
    idx = np.arange(0, S, 97)
    sub = m2[np.ix_(idx, idx)]
    expect = np.where(idx[None, :] > idx[:, None], np.float32(-1e9), 0.0)
    if not np.array_equal(sub, expect):
        return False
    full = np.triu(np.full((S, S), np.float32(-1e9)), 1)
    return np.array_equal(m2, full)


def make_in_maps(hidden_states, attention_mask, position_ids, Wq, Wk, Wv, Wo,
                 generic_mask=False):
    hs = np.asarray(hidden_states, dtype=np.float32)
    pos = np.asarray(position_ids)
    Wq = np.asarray(Wq, dtype=np.float32)
    Wk = np.asarray(Wk, dtype=np.float32)
    Wv = np.asarray(Wv, dtype=np.float32)
    Wo = np.asarray(Wo, dtype=np.float32)
    assert hs.shape == (1, S, HID)
    assert Wq.shape == (HID, HID) and Wk.shape == (HID, 1024)
    assert Wv.shape == (HID, 1024) and Wo.shape == (HID, HID)

    xT = np.ascontiguousarray(hs[0].T).astype(NPBF16)

    p = pos[0].astype(np.float32)
    inv = (1.0 / (10000.0 ** (np.arange(0, D, 2, dtype=np.float32)
                              / np.float32(D)))).astype(np.float32)
    freqs = p[:, None] * inv[None, :]
    emb = np.concatenate([freqs, freqs], axis=1)        # (S, 128)
    cosT = np.ascontiguousarray(np.cos(emb).T).astype(NPBF16)
    sinT = np.sin(emb).T.astype(np.float32)
    sinT[:64] *= np.float32(-1.0)
    # roll by 64 partitions: row p holds sign-folded sin[(p+64)%128]
    sinT = np.roll(sinT, 64, axis=0)
    sinTf = np.ascontiguousarray(sinT).astype(NPBF16)
    ones = np.ones((128, 1), dtype=NPBF16)

    if generic_mask:
        m2 = np.asarray(attention_mask, dtype=np.float32)[0, 0]
        maskT = np.stack([
            np.ascontiguousarray(
                m2[512 * (t // 4):512 * (t // 4 + 1),
                   128 * t:128 * (t + 1)].T) / np.float32(SCALE)
            for t in range(ST)
        ]).astype(NPBF16)

    in_maps = []
    for c in range(NCORES):
        wq_c = Wq[:, 512 * c:512 * (c + 1)]              # [4096, 512]
        wk_c = Wk[:, 128 * c:128 * (c + 1)]              # [4096, 128]
        wv_c = Wv[:, 128 * c:128 * (c + 1)]
        wo_c = Wo[512 * c:512 * (c + 1), :]              # [512, 4096]
        # wq_pre[g][p, kk, f] = wq_c[512g + 128kk + p, f]
        wq_pre = np.ascontiguousarray(
            wq_c.reshape(8, 4, 128, 512).transpose(0, 2, 1, 3)).astype(NPBF16)
        wkv_c = np.concatenate([wk_c, wv_c], axis=1)     # [4096, 256]
        wkv_pre = np.ascontiguousarray(
            wkv_c.reshape(8, 4, 128, 256).transpose(0, 2, 1, 3)).astype(NPBF16)
        # wo_sb[p, kh, o] = wo_c[128kh + p, o]
        wo_pre = np.ascontiguousarray(
            wo_c.reshape(4, 128, HID).transpose(1, 0, 2)).astype(NPBF16)
        m = {
            "xT": xT,
            "wq_pre": wq_pre,
            "wkv_pre": wkv_pre,
            "wo_pre": wo_pre,
            "cosT": cosT,
            "sinTf": sinTf,
            "ones": ones,
        }
        if generic_mask:
            m["maskT"] = maskT
        in_maps.append(m)
    return in_maps


def kernel(hidden_states, attention_mask, position_ids, Wq, Wk, Wv, Wo):
    os.environ["BASS_NEVER_TRACE"] = "1"
    generic = not _is_causal(attention_mask)
    in_maps = make_in_maps(hidden_states, attention_mask, position_ids,
                           Wq, Wk, Wv, Wo, generic_mask=generic)
    nc = get_nc(generic_mask=generic)
    res = run_bass_kernel_spmd(nc, in_maps, list(range(NCORES)))
    acc = np.zeros((S, HID), dtype=np.float64)
    for c in range(NCORES):
        acc += res.results[c]["y"]
    return acc.astype(np.float32)[None]


# revision 7
# speedup vs baseline: 1.0072x; 1.0009x over previous
"""Grouped-query attention (B=1, S=2048, HID=4096, 32 q-heads / 8 kv-heads,
D=128, RoPE, additive causal mask) on 8 Trainium2 NeuronCores.

Sharding (tensor-parallel over heads, per the sharding hint): core c owns 4
q-heads (columns 512c:512c+512 of Wq), kv-head c (columns 128c:128c+128 of
Wk/Wv), and rows 512c:512c+512 of Wo. Each core emits a full-shape partial
of the output projection; the host sums the 8 partials (the "all-reduce" of
the row-sharded Wo matmul).

Design notes (TimelineSim ~388us/core vs 606us baseline):
  - All matmuls bf16 (1 PE cycle/row like f32r, half the DMA/SBUF; fp8 was
    measured numerically unacceptable for the 2e-2 gate). Weights are
    host-prepacked into their exact SBUF layouts so every DMA moves large
    contiguous descriptors.
  - One unified PSUM scheme: four [128,1024] double-bank tiles (P0..P3)
    tag-rotated through warmup / QKV+RoPE / attention / output projection.
    No psum pool open/close between phases, so there are no cross-phase
    allocation barriers - a new use of a tag only waits for that tag's
    previous consumers.
  - PE warmup chain at t~0 (tiny matmuls on a zeroed tile) so the tensor
    engine p-state ramp (0.65/1.2/2.4 GHz) is complete and the PE never
    idles while the first weight/x DMAs land.
  - Causal masking is free-form: score tiles are computed at variable width
    (only columns right of the diagonal), exp'd unmasked, and the single
    boundary 128-block is multiplied by a constant lower-triangular 0/1
    bf16 mask on the DVE (2x mode). No mask DMA, no PSUM mask adds.
    A generic additive-mask fallback is built if the host detects a
    non-causal attention_mask input.
  - Attention is one software-pipelined stream of k-tile PAIRS: two k-tiles
    share a [128,1024] psum score tile and (off-diagonal) a single exp, so
    the Act engine's fixed per-op overhead is halved; lsum(ones matmul) and
    PV accumulate per tile with interleaved accumulation groups.
  - V is produced directly in [s, d] layout (x chunk as the stationary
    operand) - no PE transposes. Q/K run feature-major with RoPE applied by
    the DVE in 2x bf16 mode from an Act-drained copy of the psum (the
    sign-folded sin table is pre-rolled 64 partitions so both SBUF operands
    of each half-multiply share a base partition, a BIR requirement).
  - Softmax normalization: DVE reciprocal of the lsum row, Pool
    partition_broadcast, one DVE multiply into the bf16 outT tile.
  - Output projection drains psum via two independent [128,512] copies
    (alternating Act/DVE) so each half's y DMA fires as soon as its copy
    lands; y is written bf16 and the partials are summed f32 on the host.
"""
import os

import numpy as np
import ml_dtypes
from contextlib import ExitStack

import concourse.bass as bass
import concourse.tile as tile
from concourse import bacc, mybir
from concourse.bass_utils import run_bass_kernel_spmd

F32 = mybir.dt.float32
BF16 = mybir.dt.bfloat16
EXP = mybir.ActivationFunctionType.Exp
NPBF16 = ml_dtypes.bfloat16

S = 2048
HID = 4096
D = 128
NCORES = 8
NHQ = 4                      # q heads per core
SCALE = float(D) ** -0.5
ST = S // 128                # 16 s-tiles
SL = S // 512                # 4 s-slices
KT = HID // 128              # 32 hidden k-tiles
NO = HID // 512              # 8 output column slices
NWARM = 90                   # PE warmup matmuls (bridges first DMAs)

_NC_CACHE = {}


def build_nc(generic_mask=False, nwarm=None, debug_taps=False):
    nc = bacc.Bacc("TRN2", target_bir_lowering=False, debug=False,
                   num_devices=NCORES)
    xT = nc.dram_tensor("xT", [HID, S], BF16, kind="ExternalInput").ap()
    wq_pre = nc.dram_tensor("wq_pre", [8, 128, 4, 512], BF16,
                            kind="ExternalInput").ap()
    wkv_pre = nc.dram_tensor("wkv_pre", [8, 128, 4, 256], BF16,
                             kind="ExternalInput").ap()
    wo_pre = nc.dram_tensor("wo_pre", [128, NHQ, HID], BF16,
                            kind="ExternalInput").ap()
    cosT = nc.dram_tensor("cosT", [128, S], BF16, kind="ExternalInput").ap()
    sinTf = nc.dram_tensor("sinTf", [128, S], BF16, kind="ExternalInput").ap()
    ones = nc.dram_tensor("ones", [128, 1], BF16, kind="ExternalInput").ap()
    if generic_mask:
        maskT = nc.dram_tensor("maskT", [ST, 128, 512], BF16,
                               kind="ExternalInput").ap()
    y = nc.dram_tensor("y", [S, HID], BF16, kind="ExternalOutput").ap()
    if debug_taps:
        qt_d = nc.dram_tensor("qt_d", [NHQ, 128, S], BF16,
                              kind="ExternalOutput").ap()
        kt_d = nc.dram_tensor("kt_d", [128, S], BF16,
                              kind="ExternalOutput").ap()
        v_d = nc.dram_tensor("v_d", [128, ST, 128], BF16,
                             kind="ExternalOutput").ap()
        o_d = nc.dram_tensor("o_d", [NHQ, 128, S], BF16,
                             kind="ExternalOutput").ap()

    with tile.TileContext(nc) as tc, ExitStack() as ctx:
        const = ctx.enter_context(tc.tile_pool(name="const", bufs=1))
        cos_sb = const.tile([128, S], BF16)
        sin_sb = const.tile([128, S], BF16)
        ones_sb = const.tile([128, 1], BF16)
        qt = [const.tile([128, S], BF16, tag=f"qt{h}", name=f"qt{h}")
              for h in range(NHQ)]
        kt = const.tile([128, S], BF16)
        v_sb = const.tile([128, ST, 128], BF16)
        outT = [const.tile([128, S], BF16, tag=f"outT{h}", name=f"outT{h}")
                for h in range(NHQ)]
        tri = const.tile([128, 128], BF16)
        if generic_mask:
            mask_sb = const.tile([128, ST, 512], BF16)

        # Unified PSUM: four 2-bank [128,1024] tiles, tag-rotated through all
        # phases. No pool open/close -> no cross-phase handover barriers; a
        # new instance of tag Pn only waits for the previous Pn consumers.
        psum = ctx.enter_context(tc.tile_pool(name="psum", bufs=1,
                                              space="PSUM"))

        def ptile(tag):
            return psum.tile([128, 1024], F32, tag=tag, name=tag)

        # ---------------- PE warmup: own the clock ramp from t~0 ------------
        warm = ctx.enter_context(tc.tile_pool(name="warm", bufs=1))
        wsb = warm.tile([128, 64], BF16)
        nc.gpsimd.memset(wsb[:], 0.0)
        wps = ptile("P3")
        for _ in range(nwarm if nwarm is not None else NWARM):
            nc.tensor.matmul(wps[0:64, 0:64], wsb[:, 0:64], wsb[:, 0:64],
                             start=True, stop=True, skip_group_check=True)
        nc.gpsimd.memset(tri[:], 1.0)
        nc.gpsimd.affine_select(out=tri[:], in_=tri[:],
                                compare_op=mybir.AluOpType.is_ge, fill=0.0,
                                base=0, pattern=[[1, 128]],
                                channel_multiplier=-1)

        # ---------------- Phase A: projections + RoPE -----------------------
        wA = ctx.enter_context(tc.tile_pool(name="wA", bufs=1))
        wq_sb = wA.tile([128, KT, 512], BF16)
        wkv_sb = wA.tile([128, KT, 256], BF16)
        wo_sb = wA.tile([128, NHQ, HID], BF16)

        drains = ctx.enter_context(tc.tile_pool(name="drains", bufs=2))
        ropes = ctx.enter_context(tc.tile_pool(name="ropes", bufs=2))
        with tc.tile_pool(name="xtp", bufs=4) as xtp, \
             tc.tile_pool(name="xtp2", bufs=4) as xtp2:

            def rope_from(dst_slice, sb, cs, sn):
                """dst = sb*cos + rotate_half(sb)*sin; all-bf16 -> DVE 2x.

                sn is the sign-folded sin table rolled by 64 partitions, so
                each half-multiply reads both SBUF operands at the SAME base
                partition (BIR requires equal input bases for SB+SB ops).
                """
                rot = ropes.tile([128, 512], BF16, tag="rot")
                nc.vector.tensor_mul(rot[0:64, :], sb[64:128, :],
                                     sn[64:128, :])
                nc.vector.tensor_mul(rot[64:128, :], sb[0:64, :], sn[0:64, :])
                nc.vector.tensor_mul(dst_slice, sb[:, :], cs)
                nc.vector.tensor_add(dst_slice, dst_slice, rot[:])

            for j in range(SL):
                # P0 = [q0|q1], P1 = [q2|q3], P2 = [k|v]
                acc = [ptile("P0"), ptile("P1"), ptile("P2")]
                qps = [acc[0][:, 0:512], acc[0][:, 512:1024],
                       acc[1][:, 0:512], acc[1][:, 512:1024]]
                kps = acc[2][:, 0:512]
                vtile = acc[2][:, 512:1024]
                for g in range(8):
                    if j == 0:
                        # weights ahead of x in queue order; g=0 in halves so
                        # the first matmul group can fire earlier
                        xt = xtp.tile([128, 4, 512], BF16, tag="xt")
                        xts = xt
                        for hh in ([0, 1] if g == 0 else [None]):
                            if hh is None:
                                ksl = slice(4 * g, 4 * g + 4)
                                kk2 = slice(0, 4)
                            else:
                                ksl = slice(2 * hh, 2 * hh + 2)
                                kk2 = slice(2 * hh, 2 * hh + 2)
                            nc.sync.dma_start(out=wq_sb[:, ksl, :],
                                              in_=wq_pre[g][:, kk2, :])
                            nc.sync.dma_start(out=wkv_sb[:, ksl, :],
                                              in_=wkv_pre[g][:, kk2, :])
                            nc.sync.dma_start(
                                out=xt[:, kk2, :],
                                in_=xT[512 * g + 256 * (hh or 0):
                                       512 * g + 256 * (hh or 0) +
                                       (512 if hh is None else 256),
                                       512 * j:512 * (j + 1)]
                                .rearrange("(t p) m -> p t m", p=128))
                        if g == 7:
                            nc.sync.dma_start(out=cos_sb[:], in_=cosT[:])
                            nc.sync.dma_start(out=sin_sb[:], in_=sinTf[:])
                            nc.sync.dma_start(out=ones_sb[:], in_=ones[:])
                            if generic_mask:
                                nc.sync.dma_start(
                                    out=mask_sb[:],
                                    in_=maskT.rearrange("t p q -> p t q"))
                    else:
                        if g % 2 == 0:
                            xt = xtp2.tile([128, 8, 512], BF16, tag="xt2")
                            nc.sync.dma_start(
                                out=xt[:],
                                in_=xT[1024 * (g // 2):1024 * (g // 2 + 1),
                                       512 * j:512 * (j + 1)]
                                .rearrange("(t p) m -> p t m", p=128))
                        xts = xt[:, 4 * (g % 2):4 * (g % 2 + 1), :]
                    if j == 2 and g < 4:
                        nc.sync.dma_start(out=wo_sb[:, g, :],
                                          in_=wo_pre[:, g, :])
                    def q_mm(kk, f):
                        k = 4 * g + kk
                        nc.tensor.matmul(
                            qps[f], wq_sb[:, k, 128 * f:128 * (f + 1)],
                            xts[:, kk, :], start=(k == 0), stop=(k == KT - 1),
                            skip_group_check=True)

                    def kv_mm(kk):
                        k = 4 * g + kk
                        st, sp = (k == 0), (k == KT - 1)
                        nc.tensor.matmul(kps, wkv_sb[:, k, 0:128],
                                         xts[:, kk, :], start=st, stop=sp,
                                         skip_group_check=True)
                        # V in [s, d] layout: x chunk is the stationary
                        # side. start only on sb_==0: PSUM start zeroing is
                        # 2KB-region wide, so one start covers the whole
                        # vtile bank; per-sb_ starts would wipe each other's
                        # k=0 contribution.
                        for sb_ in range(4):
                            nc.tensor.matmul(
                                vtile[:, 128 * sb_:128 * (sb_ + 1)],
                                xts[:, kk, 128 * sb_:128 * (sb_ + 1)],
                                wkv_sb[:, k, 128:256],
                                start=st and sb_ == 0, stop=sp,
                                skip_group_check=True)

                    if g < 7:
                        for kk in range(4):
                            for f in range(NHQ):
                                q_mm(kk, f)
                        for kk in range(4):
                            kv_mm(kk)
                    elif True:
                        # last chunk: q features in drain order first, so
                        # the paired Act drains start ~4us before the j
                        # boundary and the next j's banks are free in time
                        for f in range(NHQ):
                            for kk in range(4):
                                q_mm(kk, f)
                        for kk in range(4):
                            kv_mm(kk)
                cs = cos_sb[:, 512 * j:512 * (j + 1)]
                sn = sin_sb[:, 512 * j:512 * (j + 1)]
                def drain_and_rope(j, acc, kps, vtile, cs, sn):
                    dr0 = drains.tile([128, 1024], BF16, tag="dr0",
                                      name="dr0")
                    dr1 = drains.tile([128, 1024], BF16, tag="dr1",
                                      name="dr1")
                    drk = drains.tile([128, 512], BF16, tag="drk",
                                      name="drk")
                    if j < SL - 1:
                        # paired Act drains free each 2-bank tile in one copy
                        nc.scalar.copy(dr0[:], acc[0][:])
                        nc.scalar.copy(dr1[:], acc[1][:])
                    else:
                        # last slice: 512-wide copies so phase B's first exps
                        # can slot into the Act queue between them
                        nc.scalar.copy(dr0[:, 0:512], acc[0][:, 0:512])
                        nc.scalar.copy(dr0[:, 512:1024], acc[0][:, 512:1024])
                        nc.scalar.copy(dr1[:, 0:512], acc[1][:, 0:512])
                        nc.scalar.copy(dr1[:, 512:1024], acc[1][:, 512:1024])
                    nc.scalar.copy(drk[:], kps)
                    nc.scalar.copy(v_sb[:, 4 * j:4 * (j + 1), :], vtile[:, :])
                    rope_from(qt[0][:, 512 * j:512 * (j + 1)], dr0[:, 0:512],
                              cs, sn)
                    rope_from(qt[1][:, 512 * j:512 * (j + 1)],
                              dr0[:, 512:1024], cs, sn)
                    rope_from(qt[2][:, 512 * j:512 * (j + 1)], dr1[:, 0:512],
                              cs, sn)
                    rope_from(qt[3][:, 512 * j:512 * (j + 1)],
                              dr1[:, 512:1024], cs, sn)
                    rope_from(kt[:, 512 * j:512 * (j + 1)], drk, cs, sn)

                if j < SL - 1:
                    drain_and_rope(j, acc, kps, vtile, cs, sn)
                else:
                    # defer emission of the last slice's drains into the
                    # phase-B stream (see below) so the scheduler ranks B's
                    # entry-critical exps ahead of them
                    deferred_j3 = (lambda j=j, acc=acc, kps=kps,
                                   vtile=vtile, cs=cs, sn=sn:
                                   drain_and_rope(j, acc, kps, vtile,
                                                  cs, sn))

        # ---------------- Phase B: attention --------------------------------
        # One continuous, globally software-pipelined stream of score pairs
        # across all (h, j): sc of pair i+1 overlaps exp/select of pair i,
        # including across (h, j) boundaries, so Act latency never starves PE.
        # scp pairs rotate P3/P0; the [pv | lsum] tile alternates P1/P2.
        with tc.tile_pool(name="ptbp", bufs=4) as ptbp, \
             tc.tile_pool(name="rbcp", bufs=2) as rbcp:
            work = []          # (h, j, ta, tb) in stream order
            for h in range(NHQ):
                for j in range(SL):
                    for p in range((4 * j + 4) // 2):
                        work.append((h, j, 2 * p, 2 * p + 1))

            po = {}
            pts = {}

            def emit_sc(i):
                h, j, ta, tb = work[i]
                if (h, j) not in po:
                    po[(h, j)] = ptile("P1" if (4 * h + j) % 2 == 0 else "P2")

                def off(t):
                    return max(0, 128 * (t - 4 * j))

                oa, ob = off(ta), off(tb)
                scp = ptile("P3" if i % 2 == 0 else "P0")
                nc.tensor.matmul(
                    scp[:, oa:512], kt[:, 128 * ta:128 * (ta + 1)],
                    qt[h][:, 512 * j + oa:512 * (j + 1)],
                    start=True, stop=True, skip_group_check=True)
                nc.tensor.matmul(
                    scp[:, 512 + ob:1024], kt[:, 128 * tb:128 * (tb + 1)],
                    qt[h][:, 512 * j + ob:512 * (j + 1)],
                    start=True, stop=True, skip_group_check=True)
                if generic_mask and tb >= 4 * j:
                    if ta >= 4 * j:
                        nc.vector.tensor_add(scp[:, oa:512], scp[:, oa:512],
                                             mask_sb[:, ta, oa:])
                    nc.vector.tensor_add(scp[:, 512 + ob:1024],
                                         scp[:, 512 + ob:1024],
                                         mask_sb[:, tb, ob:])
                if ta == 0 and j == 0:
                    # first pair of an (h, j0): separate tiles so ls/pv of ta
                    # only waits one 512-wide exp, not the whole pair
                    pta = ptbp.tile([128, 512], BF16, tag="ptbf",
                                    name="pta")
                    ptb2 = ptbp.tile([128, 512], BF16, tag="ptbf",
                                     name="ptb2")
                    nc.scalar.activation(pta[:, oa:], scp[:, oa:512],
                                         EXP, bias=0.0, scale=SCALE)
                    nc.scalar.activation(ptb2[:, ob:], scp[:, 512 + ob:1024],
                                         EXP, bias=0.0, scale=SCALE)
                    if not generic_mask:
                        if ta >= 4 * j:
                            nc.vector.tensor_mul(pta[:, oa:oa + 128],
                                                 pta[:, oa:oa + 128], tri[:])
                        if tb >= 4 * j:
                            nc.vector.tensor_mul(ptb2[:, ob:ob + 128],
                                                 ptb2[:, ob:ob + 128],
                                                 tri[:])
                    pts[(h, j, ta)] = pta[:, oa:]
                    pts[(h, j, tb)] = ptb2[:, ob:]
                    return
                ptb = ptbp.tile([128, 1024], BF16, tag="ptb")
                if ta >= 4 * j:          # diagonal pair: two exps
                    nc.scalar.activation(ptb[:, oa:512], scp[:, oa:512],
                                         EXP, bias=0.0, scale=SCALE)
                    nc.scalar.activation(ptb[:, 512 + ob:1024],
                                         scp[:, 512 + ob:1024], EXP,
                                         bias=0.0, scale=SCALE)
                else:                    # one exp across the pair
                    nc.scalar.activation(ptb[:, oa:1024], scp[:, oa:1024],
                                         EXP, bias=0.0, scale=SCALE)
                if not generic_mask:
                    for tx, ox, base in ((ta, oa, 0), (tb, ob, 512)):
                        if tx >= 4 * j:
                            nc.vector.tensor_mul(
                                ptb[:, base + ox:base + ox + 128],
                                ptb[:, base + ox:base + ox + 128], tri[:])
                pts[(h, j, ta)] = ptb[:, oa:512]
                pts[(h, j, tb)] = ptb[:, 512 + ob:1024]

            def emit_lspv(i):
                h, j, ta, tb = work[i]
                tmax = 4 * j + 4
                p = po[(h, j)]
                ops, lps = p[:, 0:512], p[0:1, 512:1024]
                for u in (ta, tb):
                    o = max(0, 128 * (u - 4 * j))
                    pu = pts.pop((h, j, u))
                    nc.tensor.matmul(
                        lps[:, o:512], ones_sb[:], pu,
                        start=(u == 0), stop=(u == tmax - 1),
                        skip_group_check=True)
                    nc.tensor.matmul(
                        ops[:, o:512], v_sb[:, u, :], pu,
                        start=(u == 0), stop=(u == tmax - 1),
                        skip_group_check=True)
                if tb == tmax - 1:       # (h, j) complete
                    pending_norm.append((h, j))

            def emit_norm():
                h, j = pending_norm.pop(0)
                p = po[(h, j)]
                ops, lps = p[:, 0:512], p[0:1, 512:1024]
                # stage 1/l into SBUF (DVE reciprocal can read PSUM; GPSIMD
                # cannot), broadcast on Pool, one DVE multiply
                rsb = rbcp.tile([1, 512], F32, tag="rsb")
                nc.vector.reciprocal(rsb[:], lps[:, 0:512])
                rb = rbcp.tile([128, 512], F32, tag="rb")
                nc.gpsimd.partition_broadcast(rb[:], rsb[:])
                nc.vector.tensor_mul(
                    outT[h][:, 512 * j:512 * (j + 1)], ops, rb[:])
                del po[(h, j)]

            pending_norm = []
            for i in range(len(work) + 2):
                if i == 10:
                    deferred_j3()
                if i < len(work):
                    emit_sc(i)
                if 1 <= i <= len(work):
                    emit_lspv(i - 1)
                # normalize deferred until a second one is pending (or the
                # stream ends) so its DVE ops rank behind the next
                # iteration's critical tri-mask multiplies
                while len(pending_norm) >= 2 or (i >= len(work) + 1 and
                                                 pending_norm):
                    emit_norm()

        # ---------------- Phase C: output projection ------------------------
        # two [128,512] outputs per [128,1024] psum tile
        with tc.tile_pool(name="ysbp", bufs=3) as ysbp:
            for n in range(NO):
                for sq2 in range(ST // 2):
                    yp = ptile(("P0", "P3", "P1", "P2")[sq2 % 4])
                    for half in range(2):
                        sq = 2 * sq2 + half
                        for k in range(NHQ):
                            nc.tensor.matmul(
                                yp[:, 512 * half:512 * (half + 1)],
                                outT[k][:, 128 * sq:128 * (sq + 1)],
                                wo_sb[:, k, 512 * n:512 * (n + 1)],
                                start=(k == 0), stop=(k == NHQ - 1),
                                skip_group_check=True)
                    for half in range(2):
                        sq = 2 * sq2 + half
                        ys = ysbp.tile([128, 512], BF16, tag=f"ys{half}",
                                       name=f"ys{half}")
                        if (sq2 + half) % 2 == 0:
                            nc.vector.tensor_copy(
                                ys[:], yp[:, 512 * half:512 * (half + 1)])
                        else:
                            nc.scalar.copy(
                                ys[:], yp[:, 512 * half:512 * (half + 1)])
                        nc.sync.dma_start(
                            out=y[128 * sq:128 * (sq + 1),
                                  512 * n:512 * (n + 1)],
                            in_=ys[:])

        if debug_taps:
            for h in range(NHQ):
                nc.sync.dma_start(out=qt_d[h], in_=qt[h][:])
                nc.sync.dma_start(out=o_d[h], in_=outT[h][:])
            nc.sync.dma_start(out=kt_d[:], in_=kt[:])
            nc.sync.dma_start(out=v_d[:], in_=v_sb[:])

    nc.compile()
    return nc


def get_nc(generic_mask=False):
    key = "ncg" if generic_mask else "nc"
    if key not in _NC_CACHE:
        _NC_CACHE[key] = build_nc(generic_mask)
    return _NC_CACHE[key]


def _is_causal(mask):
    m2 = np.asarray(mask, dtype=np.float32)[0, 0]
    if m2.shape != (S, S):
        return False
    # quick grid check, then full check
    idx = np.arange(0, S, 97)
    sub = m2[np.ix_(idx, idx)]
    expect = np.where(idx[None, :] > idx[:, None], np.float32(-1e9), 0.0)
    if not np.array_equal(sub, expect):
        return False
    full = np.triu(np.full((S, S), np.float32(-1e9)), 1)
    return np.array_equal(m2, full)


def make_in_maps(hidden_states, attention_mask, position_ids, Wq, Wk, Wv, Wo,
                 generic_mask=False):
    hs = np.asarray(hidden_states, dtype=np.float32)
    pos = np.asarray(position_ids)
    Wq = np.asarray(Wq, dtype=np.float32)
    Wk = np.asarray(Wk, dtype=np.float32)
    Wv = np.asarray(Wv, dtype=np.float32)
    Wo = np.asarray(Wo, dtype=np.float32)
    assert hs.shape == (1, S, HID)
    assert Wq.shape == (HID, HID) and Wk.shape == (HID, 1024)
    assert Wv.shape == (HID, 1024) and Wo.shape == (HID, HID)

    xT = np.ascontiguousarray(hs[0].T).astype(NPBF16)

    p = pos[0].astype(np.float32)
    inv = (1.0 / (10000.0 ** (np.arange(0, D, 2, dtype=np.float32)
                              / np.float32(D)))).astype(np.float32)
    freqs = p[:, None] * inv[None, :]
    emb = np.concatenate([freqs, freqs], axis=1)        # (S, 128)
    cosT = np.ascontiguousarray(np.cos(emb).T).astype(NPBF16)
    sinT = np.sin(emb).T.astype(np.float32)
    sinT[:64] *= np.float32(-1.0)
    # roll by 64 partitions: row p holds sign-folded sin[(p+64)%128]
    sinT = np.roll(sinT, 64, axis=0)
    sinTf = np.ascontiguousarray(sinT).astype(NPBF16)
    ones = np.ones((128, 1), dtype=NPBF16)

    if generic_mask:
        m2 = np.asarray(attention_mask, dtype=np.float32)[0, 0]
        maskT = np.stack([
            np.ascontiguousarray(
                m2[512 * (t // 4):512 * (t // 4 + 1),
                   128 * t:128 * (t + 1)].T) / np.float32(SCALE)
            for t in range(ST)
        ]).astype(NPBF16)

    in_maps = []
    for c in range(NCORES):
        wq_c = Wq[:, 512 * c:512 * (c + 1)]              # [4096, 512]
        wk_c = Wk[:, 128 * c:128 * (c + 1)]              # [4096, 128]
        wv_c = Wv[:, 128 * c:128 * (c + 1)]
        wo_c = Wo[512 * c:512 * (c + 1), :]              # [512, 4096]
        # wq_pre[g][p, kk, f] = wq_c[512g + 128kk + p, f]
        wq_pre = np.ascontiguousarray(
            wq_c.reshape(8, 4, 128, 512).transpose(0, 2, 1, 3)).astype(NPBF16)
        wkv_c = np.concatenate([wk_c, wv_c], axis=1)     # [4096, 256]
        wkv_pre = np.ascontiguousarray(
            wkv_c.reshape(8, 4, 128, 256).transpose(0, 2, 1, 3)).astype(NPBF16)
        # wo_sb[p, kh, o] = wo_c[128kh + p, o]
        wo_pre = np.ascontiguousarray(
            wo_c.reshape(4, 128, HID).transpose(1, 0, 2)).astype(NPBF16)
        m = {
            "xT": xT,
            "wq_pre": wq_pre,
            "wkv_pre": wkv_pre,
            "wo_pre": wo_pre,
            "cosT": cosT,
            "sinTf": sinTf,
            "ones": ones,
        }
        if generic_mask:
            m["maskT"] = maskT
        in_maps.append(m)
    return in_maps


def kernel(hidden_states, attention_mask, position_ids, Wq, Wk, Wv, Wo):
    os.environ["BASS_NEVER_TRACE"] = "1"
    generic = not _is_causal(attention_mask)
    in_maps = make_in_maps(hidden_states, attention_mask, position_ids,
                           Wq, Wk, Wv, Wo, generic_mask=generic)
    nc = get_nc(generic_mask=generic)
    res = run_bass_kernel_spmd(nc, in_maps, list(range(NCORES)))
    acc = np.zeros((S, HID), dtype=np.float64)
    for c in range(NCORES):
        acc += res.results[c]["y"]
    return acc.astype(np.float32)[None]
